# revision 24
# baseline (speedup 1.0000x reference)
"""Trainium2 Bass kernel for nn_Attention_10282151707309.

Reference computation:
  - channel LayerNorm over C=128 (biased var, eps=1e-5, affine g/b)
  - qkv = w_qkv @ xn (1x1 conv), 4 heads x 32 dims, q scaled by 1/sqrt(32)
  - full softmax attention over HW=4096 positions per (batch, head)
  - out = w_out @ attn_out + b_out

Sharding: 8 cores = (batch b in 0..3) x (spatial half in 0..1).
Each core runs an IDENTICAL program; per-core inputs differ:
  - x is the batch slice, spatially rolled so the core's own 2048 query
    columns are always program-columns 0:2048 (attention is permutation-
    equivariant over key positions, so the roll is harmless).
  - every core computes LN + k/v over all 4096 positions of its batch
    (2x redundant per batch, cheap) and q only over its own half.
No collectives; each core writes a disjoint slice of the output.

Performance model (from the baseline's perfetto trace):
  - PE streams 512-col bf16 matmuls at 215ns issue-to-issue (2.4GHz, one
    column/cycle) with LDWEIGHTS fully hidden under the previous matmul.
    sim+av = 524288 columns ~ 220us: the PE has slack.
  - The ACT engine is the wall: exp of 33.5M sim elements at 128 lanes
    @1.2GHz with ~350 cycles fixed cost per ACTIVATE. Everything else
    must stay off ACT and the exp ops must be as large as PSUM allows.
Design consequences:
  - One activation-table preload (natural_log_exp_and_others) up front;
    Ln/Exp both live there, so zero ACT_TABLE_LOADs in steady state
    (the baseline lost 52us to 41 of them).
  - Tail softmax 1/denominator on DVE (reciprocal), not ACT ln/exp.
    All PSUM->SBUF copies and bias adds on DVE; x^2/casts on GPSIMD.
  - k-bias is dropped entirely: sim_h[j,i] += q_h[:,i]@bias_k is
    constant over j, and softmax over j is shift-invariant. Exact.
  - LN + projections are EMISSION-INTERLEAVED with i-tile 0's attention:
    LN s-tile t's stats/bc/proj slot between jc groups, so the exp
    stream starts ~6us into the kernel instead of after a 130us LN
    phase. LN keeps a 2-PSUM-bank footprint (one [128,1024] buffer
    cycled stats -> bc -> q|k proj -> v proj).
  - All sim/av matmuls are full-mode (128,128) tiles -- partial-tile
    matmuls measured ~1.6x slower per column and break the PE pipeline.
    sim avoids a padded k by using the whole k_sb chunk as lhsT with
    per-head ZERO-PADDED q copies as rhs (zeros kill the other heads'
    k rows exactly); av uses the baseline's zero-padded vaug tiles.
PSUM budget (8 banks of 2KB/partition):
  lnps [128,1024] (2, also the tails' bc|y) + duo 2x[128,1024] (4)
  + pairs 2x[128,512] (2)
"""

import numpy as np

HEADS = 4
DIM_HEAD = 32
B, C, H, W = 4, 128, 64, 64
S = H * W              # 4096 spatial positions
HALF = S // 2          # 2048 own query columns per core
TI = 512               # i-tile (query) size
NIT = HALF // TI       # 4 i-tiles
JCHUNK = 128           # j-chunk (key) size
NJC = S // JCHUNK      # 32 j-chunks
EPS = 1e-5
N_CORES = 8
VW = DIM_HEAD + 1      # 33: v dims + softmax-denominator ones column

_PROGRAM = None


def _build_program():
    """Build the (SPMD-identical) Bass program once per process."""
    import concourse.bass as bass  # noqa: F401
    import concourse.mybir as mybir
    import concourse.tile as tile
    from concourse import bacc
    from concourse.bass import ts

    dt = mybir.dt.float32
    dtb = mybir.dt.bfloat16
    F = mybir.ActivationFunctionType
    Op = mybir.AluOpType

    nc = bacc.Bacc(
        "TRN2",
        target_bir_lowering=False,
        debug=False,
        num_devices=N_CORES,
    )

    x_d = nc.dram_tensor("x", [C, S], dt, kind="ExternalInput").ap()
    wq_d = nc.dram_tensor("wq_t", [C, 128], dtb, kind="ExternalInput").ap()
    wk_d = nc.dram_tensor("wk_t", [C, 128], dtb, kind="ExternalInput").ap()
    wv_d = nc.dram_tensor("wv_t", [C, 128], dtb, kind="ExternalInput").ap()
    woa_d = nc.dram_tensor("wo_a", [97, 128], dt, kind="ExternalInput").ap()
    wob_d = nc.dram_tensor("wo_b", [97, 128], dt, kind="ExternalInput").ap()
    bq_d = nc.dram_tensor("bias_q", [128, 1], dt, kind="ExternalInput").ap()
    bo_d = nc.dram_tensor("bias_o", [128, 1], dt, kind="ExternalInput").ap()
    y_d = nc.dram_tensor("y", [C, HALF], dt, kind="ExternalOutput").ap()

    with tile.TileContext(nc) as tc:
        from contextlib import ExitStack

        with ExitStack() as ctx:
            const_pool = ctx.enter_context(tc.tile_pool(name="const", bufs=1))
            big_pool = ctx.enter_context(tc.tile_pool(name="big", bufs=1))

            # One table set (natural_log_exp_and_others, id 6) serves every
            # activation in this kernel (Exp, Ln); preload it once so the
            # table-placement pass never ping-pongs between the exp-only and
            # ln-only tables.
            nc.scalar.add_instruction(
                mybir.InstLoadActFuncSet(
                    name="act_preload", act_func_set_id=6, ins=[], outs=[]
                )
            )

            wq = const_pool.tile([C, 128], dtb, tag="wq")
            wk = const_pool.tile([C, 128], dtb, tag="wk")
            wv = const_pool.tile([C, 128], dtb, tag="wv")
            woa = const_pool.tile([97, 128], dt, tag="woa")
            wob = const_pool.tile([97, 128], dt, tag="wob")
            bq = const_pool.tile([128, 1], dt, tag="bq")
            bo = const_pool.tile([128, 1], dt, tag="bo")
            ones1 = const_pool.tile([1, 128], dt, tag="ones1")
            onesC = const_pool.tile([128, 1], dtb, tag="onesC")
            # bc lhsT: row 0 selects the even-head reciprocal into out rows
            # 0:33, row 32 the odd-head one into rows 64:97 (engine AP
            # partition bases must be 32-aligned, so the two reciprocal rows
            # live at partitions 0 and 32; rows 1:32 are zero against
            # whatever sits in the rec tile there).
            ones2 = const_pool.tile([33, 97], dt, tag="ones2")
            epsc = const_pool.tile([1, 1], dt, tag="epsc")

            x_sb = big_pool.tile([C, S], dt, tag="x")
            xn = big_pool.tile([C, S], dtb, tag="xn")
            k_sb = big_pool.tile([128, S], dtb, tag="k")
            # vaug: per j-chunk four [128, 128] full-mode av lhsT tiles in
            # order [h0, h2, h1, h3]. Tiles for h0/h1 carry (v^T | ones) at
            # cols 0-32; tiles for h2/h3 at cols 64-96; everything else zero,
            # so each matmul writes only its head's rows of the pair bank.
            # Full-mode (128,128) tiles: partial-tile matmuls stream at half
            # rate / break the PE pipeline (measured 601ns vs 377ns per 512
            # columns), so both sim and av stay (128,128).
            vaug = big_pool.tile([128, NJC * 512], dtb, tag="vaug")
            # sim full-mode without a padded k: lhsT is the whole k_sb chunk
            # (all 4 heads' rows); the ZERO-PADDED per-head q copy selects
            # head h (rows outside 32h:32h+32 are zero, killing the other
            # heads' k rows exactly).
            q_pad = [
                big_pool.tile(
                    [128, HALF], dtb, tag=f"qpad{h}", name=f"qpad{h}"
                )
                for h in range(HEADS)
            ]
            catA = big_pool.tile([128, TI], dt, tag="catA")
            catB = big_pool.tile([128, TI], dt, tag="catB")
            recA = big_pool.tile([33, TI], dt, tag="recA")
            recB = big_pool.tile([33, TI], dt, tag="recB")

            # input DMAs: x s-tile 0 first (heads the LN pipeline), then the
            # small weights, then the rest of x.
            nc.sync.dma_start(x_sb[:, 0:512], x_d[:, 0:512])
            nc.sync.dma_start(wq[:], wq_d[:])
            nc.sync.dma_start(wk[:], wk_d[:])
            nc.sync.dma_start(wv[:], wv_d[:])
            nc.sync.dma_start(woa[:], woa_d[:])
            nc.sync.dma_start(wob[:], wob_d[:])
            nc.sync.dma_start(bq[:], bq_d[:])
            nc.sync.dma_start(bo[:], bo_d[:])
            for t in range(1, 8):
                nc.sync.dma_start(x_sb[:, ts(t, 512)], x_d[:, ts(t, 512)])

            nc.vector.memset(ones1[:], 1.0)
            nc.vector.memset(onesC[:], 1.0 / C)
            nc.vector.memset(ones2[:], 0.0)
            nc.vector.memset(ones2[0:1, 0:33], 1.0)
            nc.vector.memset(ones2[32:33, 64:97], 1.0)
            nc.vector.memset(recA[:], 0.0)
            nc.vector.memset(recB[:], 0.0)
            nc.vector.memset(epsc[:], EPS)
            # zero fills: q pads on gpsimd, vaug split between DVE (first
            # chunk, needed early by jc 0-7's v copies) and gpsimd.
            for h in range(HEADS):
                nc.gpsimd.memset(q_pad[h][:], 0.0)
            nc.vector.memset(vaug[:, 0 : 8 * 512], 0.0)
            nc.gpsimd.memset(vaug[:, 8 * 512 :], 0.0)
            # softmax-denominator ones columns (cols 32 of the h0/h1 tiles,
            # cols 96 of the h2/h3 tiles)
            ones_even = vaug[:].rearrange(
                "p (c g e) -> p c g e", g=2, e=256
            )[:, :, :, 32:33]
            nc.vector.memset(ones_even, 1.0)
            ones_odd = vaug[:].rearrange(
                "p (c g e) -> p c g e", g=2, e=256
            )[:, :, :, 224:225]
            nc.vector.memset(ones_odd, 1.0)
            # cat rows 33:64 are never written but are read by the K=97
            # y matmul (against zero rows of wo) -- must not be NaN.
            # (row 32 is rewritten by every tail; zeroing from 32 keeps the
            # memset partition base 32-aligned.)
            nc.vector.memset(catA[32:64, :], 0.0)
            nc.vector.memset(catB[32:64, :], 0.0)

            pair_pool = ctx.enter_context(
                tc.tile_pool(name="pair_ps", bufs=2, space="PSUM")
            )
            expo_pool = ctx.enter_context(tc.tile_pool(name="expo", bufs=3))
            rec_pool = ctx.enter_context(tc.tile_pool(name="rec", bufs=2))
            ysb_pool = ctx.enter_context(tc.tile_pool(name="ysb", bufs=2))
            sm_pool = ctx.enter_context(tc.tile_pool(name="lnsm", bufs=2))
            gx_pool = ctx.enter_context(tc.tile_pool(name="lngx", bufs=2))

            # ---------------- LayerNorm + projections ----------------
            # Per s-tile (512 positions), using one [128,1024] PSUM buffer
            # cycled through 4 generations: stats -> bc -> (qp|kp) -> vp.
            # Emission is split into three slots (A/B/C) that interleave
            # with i-tile 0's attention groups.
            ln_state = {}

            def emit_ln_A(lnps, t):
                sl = ts(t, 512)
                g = lnps.tile([128, 1024], dt, tag="ln")
                xb = gx_pool.tile([128, 512], dtb, tag="xb")
                xsq = gx_pool.tile([128, 512], dtb, tag="xsq")
                nc.gpsimd.tensor_copy(xb[:], x_sb[:, sl])
                nc.gpsimd.tensor_tensor(xsq[:], x_sb[:, sl], x_sb[:, sl], Op.mult)
                nc.tensor.matmul(g[0:1, 0:512], onesC[:, 0:1], xb[:])
                nc.tensor.matmul(g[0:1, 512:1024], onesC[:, 0:1], xsq[:])
                ln_state[t] = g

            def emit_ln_B(t):
                g = ln_state[t]
                # mean to SBUF first: DVE ops may read at most one PSUM
                # operand (single PSUM read port).
                mcp = sm_pool.tile([1, 512], dt, tag="mcp")
                msq = sm_pool.tile([1, 512], dt, tag="msq")
                var = sm_pool.tile([1, 512], dt, tag="var")
                lnv = sm_pool.tile([1, 512], dt, tag="lnv")
                ru = sm_pool.tile([1, 1024], dt, tag="ru")
                nc.vector.tensor_copy(mcp[:], g[0:1, 0:512])
                nc.vector.tensor_tensor(msq[:], mcp[:], mcp[:], Op.mult)
                nc.vector.scalar_tensor_tensor(
                    var[:], g[0:1, 512:1024], 1.0, msq[:], Op.mult, Op.subtract
                )
                nc.scalar.activation(lnv[:], var[:], F.Ln, bias=epsc[0:1, 0:1])
                nc.scalar.activation(ru[0:1, 0:512], lnv[:], F.Exp, scale=-0.5)
                # u = mean * rstd
                nc.vector.tensor_tensor(
                    ru[0:1, 512:1024], mcp[:], ru[0:1, 0:512], Op.mult
                )
                ln_state[t] = ru

            def emit_ln_C(lnps, t):
                sl = ts(t, 512)
                ru = ln_state.pop(t)
                bc = lnps.tile([128, 1024], dt, tag="ln")
                nc.tensor.matmul(bc[:, 0:512], ones1[0:1, :], ru[0:1, 0:512])
                nc.tensor.matmul(bc[:, 512:1024], ones1[0:1, :], ru[0:1, 512:1024])
                tmp = gx_pool.tile([128, 512], dt, tag="xtmp")
                nc.vector.tensor_tensor(tmp[:], x_sb[:, sl], bc[:, 0:512], Op.mult)
                nc.vector.tensor_tensor(
                    xn[:, sl], tmp[:], bc[:, 512:1024], Op.subtract
                )
                qk = lnps.tile([128, 1024], dt, tag="ln")
                if t < NIT:
                    nc.tensor.matmul(qk[:, 0:512], wq[:], xn[:, sl])
                    for h in range(HEADS):
                        nc.vector.tensor_scalar(
                            q_pad[h][32 * h : 32 * h + 32, sl],
                            qk[32 * h : 32 * h + 32, 0:512],
                            bq[32 * h : 32 * h + 32, 0:1],
                            None,
                            Op.add,
                        )
                nc.tensor.matmul(qk[:, 512:1024], wk[:], xn[:, sl])
                nc.vector.tensor_copy(k_sb[:, sl], qk[:, 512:1024])
                vp = lnps.tile([128, 1024], dt, tag="ln")
                for c in range(4):
                    jc = 4 * t + c
                    nc.tensor.matmul(
                        vp[:, 128 * c : 128 * c + 128], xn[:, ts(jc, 128)], wv[:]
                    )
                    base = jc * 512
                    # h0 -> tile 0 cols 0:32, h1 -> tile 2 cols 0:32
                    dst01 = (
                        vaug[:, base : base + 512]
                        .rearrange("p (g e) -> p g e", e=256)[:, :, 0:32]
                    )
                    src01 = vp[:, 128 * c : 128 * c + 128].rearrange(
                        "p (g e) -> p g e", e=32
                    )[:, 0:2, :]
                    nc.vector.tensor_copy(dst01, src01)
                    # h2 -> tile 1 cols 64:96, h3 -> tile 3 cols 64:96
                    dst23 = (
                        vaug[:, base : base + 512]
                        .rearrange("p (g e) -> p g e", e=256)[:, :, 192:224]
                    )
                    src23 = vp[:, 128 * c : 128 * c + 128].rearrange(
                        "p (g e) -> p g e", e=32
                    )[:, 2:4, :]
                    nc.vector.tensor_copy(dst23, src23)

            # ---------------- attention ----------------
            # Baseline-proven full-mode structure: per (it, jc) two duo
            # tiles (X = heads 0|2, Y = heads 1|3), two N=1024 exps, four
            # full-mode av matmuls. The software pipeline keeps the PE
            # streaming (215ns per 512-col matmul) and the ACT exp stream
            # back-to-back. The i-tile tail is emitted INTERLEAVED into the
            # next i-tile's first jc iterations so its latency (DVE
            # reciprocals, bc/y matmuls) hides under the exp stream.
            def emit_tail(it, pairA, pairB):
                isl = ts(it, TI)
                bcy = lnps_pool.tile([128, 1024], dt, tag="ln")
                bc = bcy[0:97, 0:512]
                yp = bcy[0:128, 512:1024]
                # both reciprocal pairs first so the DVE chain pipelines
                nc.vector.reciprocal(recA[0:1, :], pairA[32:33, :])
                nc.vector.reciprocal(recA[32:33, :], pairA[96:97, :])
                nc.vector.reciprocal(recB[0:1, :], pairB[32:33, :])
                nc.vector.reciprocal(recB[32:33, :], pairB[96:97, :])
                for pi, (pair, cat, rec, wo) in enumerate(
                    ((pairA, catA, recA, woa), (pairB, catB, recB, wob))
                ):
                    bcs = rec_pool.tile([97, 512], dt, tag="bcs")
                    nc.tensor.matmul(bc, ones2[0:33, :], rec[0:33, :])
                    # bc to SBUF: cat = pair * bc would be two PSUM reads
                    nc.vector.tensor_copy(bcs[0:33, :], bc[0:33, :])
                    nc.vector.tensor_copy(bcs[64:97, :], bc[64:97, :])
                    nc.vector.tensor_tensor(
                        cat[0:33, :], pair[0:33, :], bcs[0:33, :], Op.mult
                    )
                    nc.vector.tensor_tensor(
                        cat[64:97, :], pair[64:97, :], bcs[64:97, :], Op.mult
                    )
                    nc.tensor.matmul(
                        yp, wo[:, :], cat[0:97, :], start=pi == 0, stop=pi == 1
                    )
                ysb = ysb_pool.tile([128, TI], dt, tag="ysb")
                nc.vector.tensor_scalar(ysb[:], yp, bo[:, 0:1], None, Op.add)
                nc.sync.dma_start(y_d[:, isl], ysb[:])

            def run_it(it, duop, ln_interleave):
                isl = ts(it, TI)
                pairA = pair_pool.tile([128, TI], dt, tag="pair")
                pairB = pair_pool.tile([128, TI], dt, tag="pair")

                def emit_sims(jc):
                    ksl = ts(jc, JCHUNK)
                    duoX = duop.tile([128, 1024], dt, tag="duo")
                    nc.tensor.matmul(duoX[:, 0:512], k_sb[:, ksl], q_pad[0][:, isl])
                    nc.tensor.matmul(duoX[:, 512:1024], k_sb[:, ksl], q_pad[2][:, isl])
                    duoY = duop.tile([128, 1024], dt, tag="duo")
                    nc.tensor.matmul(duoY[:, 0:512], k_sb[:, ksl], q_pad[1][:, isl])
                    nc.tensor.matmul(duoY[:, 512:1024], k_sb[:, ksl], q_pad[3][:, isl])
                    return duoX, duoY

                duoX, duoY = emit_sims(0)
                for jc in range(NJC):
                    st, sp_ = jc == 0, jc == NJC - 1
                    vbase = jc * 512
                    if ln_interleave and jc % 4 and jc // 4 + 1 < 8:
                        t = jc // 4 + 1
                        if jc % 4 == 1:
                            emit_ln_A(lnps_pool, t)
                        elif jc % 4 == 2:
                            emit_ln_B(t)
                        else:
                            emit_ln_C(lnps_pool, t)
                    expX = expo_pool.tile([128, 1024], dtb, tag="expo")
                    nc.scalar.activation(expX[:], duoX[:], F.Exp)
                    expY = expo_pool.tile([128, 1024], dtb, tag="expo")
                    nc.scalar.activation(expY[:], duoY[:], F.Exp)
                    if jc + 1 < NJC:
                        duoX, duoY = emit_sims(jc + 1)
                    nc.tensor.matmul(
                        pairA[:, :], vaug[:, vbase : vbase + 128],
                        expX[:, 0:512],
                        start=st, stop=False, skip_group_check=True,
                    )
                    nc.tensor.matmul(
                        pairA[:, :], vaug[:, vbase + 128 : vbase + 256],
                        expX[:, 512:1024],
                        start=False, stop=sp_, skip_group_check=True,
                    )
                    nc.tensor.matmul(
                        pairB[:, :], vaug[:, vbase + 256 : vbase + 384],
                        expY[:, 0:512],
                        start=st, stop=False, skip_group_check=True,
                    )
                    nc.tensor.matmul(
                        pairB[:, :], vaug[:, vbase + 384 : vbase + 512],
                        expY[:, 512:1024],
                        start=False, stop=sp_, skip_group_check=True,
                    )
                emit_tail(it, pairA, pairB)

            lnps_pool = ctx.enter_context(
                tc.tile_pool(name="lnps", bufs=1, space="PSUM")
            )
            duop = ctx.enter_context(
                tc.tile_pool(name="duo", bufs=2, space="PSUM")
            )
            emit_ln_A(lnps_pool, 0)
            emit_ln_B(0)
            emit_ln_C(lnps_pool, 0)
            for it in range(NIT):
                run_it(it, duop, it == 0)

    nc.compile()
    return nc


def _get_program():
    global _PROGRAM
    if _PROGRAM is None:
        _PROGRAM = _build_program()
    return _PROGRAM


def _prep_inputs(x, g, b, w_qkv, w_out, b_out):
    """Host-side sharding + weight folding. All tiny except x slicing."""
    f32 = np.float32
    x = np.asarray(x, f32).reshape(B, C, S)
    g_ = np.asarray(g, f32).reshape(C)
    b_ = np.asarray(b, f32).reshape(C)
    w_qkv = np.asarray(w_qkv, f32)
    w_out = np.asarray(w_out, f32)
    b_out = np.asarray(b_out, f32)

    import ml_dtypes

    bf16 = ml_dtypes.bfloat16
    scale = DIM_HEAD ** -0.5
    wg = w_qkv * g_[None, :]
    bias_qkv = w_qkv @ b_
    hid = HEADS * DIM_HEAD  # 128
    wq_t = np.ascontiguousarray((wg[0:hid] * scale).T).astype(bf16)
    wk_t = np.ascontiguousarray(wg[hid : 2 * hid].T).astype(bf16)
    wv_t = np.ascontiguousarray(wg[2 * hid : 3 * hid].T).astype(bf16)
    bias_q = np.ascontiguousarray((bias_qkv[0:hid] * scale).reshape(128, 1))
    # bias_k is dropped: it shifts all logits of a query equally, and
    # softmax is shift-invariant. bias_v folds exactly into the output
    # bias (attention rows sum to 1).
    bias_v = bias_qkv[2 * hid : 3 * hid]

    wo_t = w_out.T  # [hd, o]
    wo_a = np.zeros((97, 128), f32)
    wo_b = np.zeros((97, 128), f32)
    wo_a[0:32] = wo_t[0:32]     # head 0
    wo_a[64:96] = wo_t[64:96]   # head 2
    wo_b[0:32] = wo_t[32:64]    # head 1
    wo_b[64:96] = wo_t[96:128]  # head 3
    bias_o = np.ascontiguousarray((b_out + w_out @ bias_v).reshape(128, 1))

    shared = {
        "wq_t": wq_t,
        "wk_t": wk_t,
        "wv_t": wv_t,
        "wo_a": wo_a,
        "wo_b": wo_b,
        "bias_q": bias_q,
        "bias_o": bias_o,
    }
    in_maps = []
    for core in range(N_CORES):
        bb, half = core // 2, core % 2
        if half == 0:
            xc = x[bb]
        else:
            xc = np.concatenate([x[bb][:, HALF:], x[bb][:, :HALF]], axis=1)
        m = {"x": np.ascontiguousarray(xc)}
        m.update(shared)
        in_maps.append(m)
    return in_maps


def _run(inputs, trace=False):
    from concourse.bass_utils import run_bass_kernel_spmd

    nc = _get_program()
    in_maps = _prep_inputs(**inputs)
    res = run_bass_kernel_spmd(
        nc, in_maps, core_ids=list(range(N_CORES)), trace=trace
    )
    y = np.empty((B, C, S), np.float32)
    for core in range(N_CORES):
        bb, half = core // 2, core % 2
        yc = res.results[core]["y"]
        if half == 0:
            y[bb][:, :HALF] = yc
        else:
            y[bb][:, HALF:] = yc
    return y.reshape(B, C, H, W), res


def kernel(x, g, b, w_qkv, w_out, b_out):
    out, _ = _run(
        {"x": x, "g": g, "b": b, "w_qkv": w_qkv, "w_out": w_out, "b_out": b_out}
    )
    return out


# revision 29
# speedup vs baseline: 1.3048x; 1.3048x over previous
"""Trainium2 Bass kernel for nn_Attention_10282151707309.

Reference computation:
  - channel LayerNorm over C=128 (biased var, eps=1e-5, affine g/b)
  - qkv = w_qkv @ xn (1x1 conv), 4 heads x 32 dims, q scaled by 1/sqrt(32)
  - full softmax attention over HW=4096 positions per (batch, head)
  - out = w_out @ attn_out + b_out

Sharding: 8 cores = (batch b in 0..3) x (spatial half in 0..1); each core
runs an identical program on its batch slice (spatially rolled so its own
2048 query columns are program-columns 0:2048 -- softmax is permutation-
equivariant over keys). No collectives; disjoint output slices.

Performance model (from perfetto traces of earlier versions):
  - The ACT engine is the wall: exp of 33.5M sim values at 128 lanes
    @1.2GHz, ~(N+390)/1.2 ns per N-element ACTIVATE. Steady state
    achieves one [128,1024] exp per ~1.2us; everything else must stay
    off ACT and off the exp stream's critical path.
  - PE streams full-mode (128,128)-tile bf16 matmuls at ~215-258ns per
    512 columns with LDWEIGHTS hidden; partial-tile matmuls are ~1.6x
    slower, so sim/av use only full [128,x] operands.
  - Engine queues are strictly in-order: any op whose producers are not
    long-finished head-of-line blocks its whole engine. All cross-engine
    chains (LN, tails) are therefore emission-scheduled several jc
    iterations after their producers.
Structure:
  - One activation-table preload (natural_log_exp_and_others) so Ln/Exp
    never swap tables (the original baseline lost 52us to 41 loads).
  - LN + projections are emission-interleaved under i-tile 0's attention
    in 5 slots per s-tile (xb/xsq -> stats -> rstd -> bc/xn -> proj),
    each ~2 jc after its producers. s-tiles 0-1 partially in prologue.
  - sim full-mode without a padded k: lhsT is the whole k_sb chunk (all
    4 heads' rows), rhs a per-head ZERO-PADDED q copy (zeros kill the
    other heads' k rows exactly). q pads and the zero-padded av lhsT
    tiles (vaug) are zero-filled by DMA from a host zeros tensor --
    no big on-chip memsets (gpsimd per-op overhead is ~1.2-2.2us).
  - k-bias dropped entirely (softmax shift-invariance, exact); v-bias
    folded into the output bias; g/b folded into the qkv weights.
  - i-tile tails are taken OFF the critical path: pairs are spilled
    PSUM->SBUF (2 DVE copies) at it end, which immediately frees the
    pair banks for the next i-tile; the normalize/project tail runs
    against the SBUF copy, sprinkled into the next i-tile's jc loop.
PSUM (8 banks): lnps [128,1024] (2; LN generations + tail bc|y)
  + duo 2x[128,1024] (4) + pairs 2x[128,512] (2).
"""

import numpy as np

HEADS = 4
DIM_HEAD = 32
B, C, H, W = 4, 128, 64, 64
S = H * W              # 4096 spatial positions
HALF = S // 2          # 2048 own query columns per core
TI = 512               # i-tile (query) size
NIT = HALF // TI       # 4 i-tiles
JCHUNK = 128           # j-chunk (key) size
NJC = S // JCHUNK      # 32 j-chunks
EPS = 1e-5
N_CORES = 8

_PROGRAM = None


def _build_program():
    """Build the (SPMD-identical) Bass program once per process."""
    import concourse.bass as bass  # noqa: F401
    import concourse.mybir as mybir
    import concourse.tile as tile
    from concourse import bacc
    from concourse.bass import ts

    dt = mybir.dt.float32
    dtr = mybir.dt.float32r
    dtb = mybir.dt.bfloat16
    F = mybir.ActivationFunctionType
    Op = mybir.AluOpType

    nc = bacc.Bacc(
        "TRN2",
        target_bir_lowering=False,
        debug=False,
        num_devices=N_CORES,
    )

    x_d = nc.dram_tensor("x", [C, S], dt, kind="ExternalInput").ap()
    wq_d = nc.dram_tensor("wq_t", [C, 128], dtb, kind="ExternalInput").ap()
    wk_d = nc.dram_tensor("wk_t", [C, 128], dtb, kind="ExternalInput").ap()
    wv_d = nc.dram_tensor("wv_t", [C, 128], dtb, kind="ExternalInput").ap()
    woa_d = nc.dram_tensor("wo_a", [97, 128], dt, kind="ExternalInput").ap()
    wob_d = nc.dram_tensor("wo_b", [97, 128], dt, kind="ExternalInput").ap()
    bq_d = nc.dram_tensor("bias_q", [128, 1], dt, kind="ExternalInput").ap()
    bo_d = nc.dram_tensor("bias_o", [128, 1], dt, kind="ExternalInput").ap()
    z_d = nc.dram_tensor("zeros", [128, 4096], dtb, kind="ExternalInput").ap()
    y_d = nc.dram_tensor("y", [C, HALF], dt, kind="ExternalOutput").ap()

    with tile.TileContext(nc) as tc:
        from contextlib import ExitStack

        with ExitStack() as ctx:
            const_pool = ctx.enter_context(tc.tile_pool(name="const", bufs=1))
            big_pool = ctx.enter_context(tc.tile_pool(name="big", bufs=1))

            # One table set (natural_log_exp_and_others, id 6) serves every
            # activation in this kernel (Exp, Ln); preload it once.
            nc.scalar.add_instruction(
                mybir.InstLoadActFuncSet(
                    name="act_preload", act_func_set_id=6, ins=[], outs=[]
                )
            )

            wq = const_pool.tile([C, 128], dtb, tag="wq")
            wk = const_pool.tile([C, 128], dtb, tag="wk")
            wv = const_pool.tile([C, 128], dtb, tag="wv")
            woa = const_pool.tile([97, 128], dt, tag="woa")
            wob = const_pool.tile([97, 128], dt, tag="wob")
            bq = const_pool.tile([128, 1], dt, tag="bq")
            bo = const_pool.tile([128, 1], dt, tag="bo")
            ones1 = const_pool.tile([1, 128], dtb, tag="ones1")
            onesC = const_pool.tile([128, 1], dtb, tag="onesC")
            # bc lhsT: row 0 -> out rows 0:33 (even-head reciprocal), row 32
            # -> out rows 64:97 (odd-head). Engine AP partition bases must be
            # 32-aligned, so the two reciprocal rows sit at partitions 0/32.
            ones2 = const_pool.tile([33, 97], dt, tag="ones2")
            epsc = const_pool.tile([1, 1], dt, tag="epsc")

            x_sb = big_pool.tile([C, S], dt, tag="x")
            xn = big_pool.tile([C, S], dtb, tag="xn")
            k_sb = big_pool.tile([128, S], dtb, tag="k")
            # vaug: per j-chunk four [128, 128] full-mode av lhsT tiles in
            # order [h0, h2, h1, h3]; h0/h1 carry (v^T | ones) at cols 0-32,
            # h2/h3 at cols 64-96, everything else zero.
            vaug = big_pool.tile([128, NJC * 512], dtb, tag="vaug")
            q_pad = [
                big_pool.tile(
                    [128, HALF], dtb, tag=f"qpad{h}", name=f"qpad{h}"
                )
                for h in range(HEADS)
            ]
            catA = big_pool.tile([128, TI], dt, tag="catA")
            catB = big_pool.tile([128, TI], dt, tag="catB")
            rec = big_pool.tile([64, 1024], dt, tag="rec")

            # input DMAs; zero fills come from the host zeros tensor so no
            # engine spends time on them.
            nc.sync.dma_start(x_sb[:, 0:512], x_d[:, 0:512])
            nc.sync.dma_start(wq[:], wq_d[:])
            nc.sync.dma_start(wk[:], wk_d[:])
            nc.sync.dma_start(wv[:], wv_d[:])
            nc.sync.dma_start(woa[:], woa_d[:])
            nc.sync.dma_start(wob[:], wob_d[:])
            nc.sync.dma_start(bq[:], bq_d[:])
            nc.sync.dma_start(bo[:], bo_d[:])
            nc.sync.dma_start(x_sb[:, 512:1024], x_d[:, 512:1024])
            for h in range(HEADS):
                nc.sync.dma_start(q_pad[h][:], z_d[:, 0:HALF])
            for t in range(2, 4):
                nc.sync.dma_start(x_sb[:, ts(t, 512)], x_d[:, ts(t, 512)])
            for i in range(4):
                nc.sync.dma_start(
                    vaug[:, 4096 * i : 4096 * (i + 1)], z_d[:, 0:4096]
                )
                if 4 + i < 8:
                    nc.sync.dma_start(
                        x_sb[:, ts(4 + i, 512)], x_d[:, ts(4 + i, 512)]
                    )

            nc.vector.memset(ones1[:], 1.0)
            nc.vector.memset(onesC[:], 1.0 / C)
            nc.vector.memset(ones2[:], 0.0)
            nc.vector.memset(ones2[0:1, 0:33], 1.0)
            nc.vector.memset(ones2[32:33, 64:97], 1.0)
            nc.vector.memset(rec[:], 0.0)
            nc.vector.memset(epsc[:], EPS)
            # softmax-denominator ones columns (col 32 of h0/h1 tiles,
            # col 96 of h2/h3 tiles)
            ones_even = vaug[:].rearrange(
                "p (c g e) -> p c g e", g=2, e=256
            )[:, :, :, 32:33]
            nc.vector.memset(ones_even, 1.0)
            ones_odd = vaug[:].rearrange(
                "p (c g e) -> p c g e", g=2, e=256
            )[:, :, :, 224:225]
            nc.vector.memset(ones_odd, 1.0)
            # cat rows 33:64 are read by the K=97 y matmul (against zero
            # rows of wo) -- zero once so they are never NaN. Row 32 is
            # rewritten by every tail (32-aligned memset base).
            nc.vector.memset(catA[32:64, :], 0.0)
            nc.vector.memset(catB[32:64, :], 0.0)

            pair_pool = ctx.enter_context(
                tc.tile_pool(name="pair_ps", bufs=2, space="PSUM")
            )
            lnps_pool = ctx.enter_context(
                tc.tile_pool(name="lnps", bufs=1, space="PSUM")
            )
            duop = ctx.enter_context(
                tc.tile_pool(name="duo", bufs=2, space="PSUM")
            )
            expo_pool = ctx.enter_context(tc.tile_pool(name="expo", bufs=6))
            spill_pool = ctx.enter_context(tc.tile_pool(name="spill", bufs=2))
            ysb_pool = ctx.enter_context(tc.tile_pool(name="ysb", bufs=2))
            sm_pool = ctx.enter_context(tc.tile_pool(name="lnsm", bufs=2))
            gx_pool = ctx.enter_context(tc.tile_pool(name="lngx", bufs=3))

            # ---------------- LayerNorm + projections ----------------
            # Five emission slots per s-tile; in i-tile 0 they sit ~2 jc
            # after their producers so no engine head-of-line blocks.
            ln_state = {}

            def ln_A(t):
                sl = ts(t, 512)
                xb = gx_pool.tile([128, 512], dtb, tag="xb")
                xsq = gx_pool.tile([128, 512], dtb, tag="xsq")
                nc.vector.tensor_copy(xb[:], x_sb[:, sl])
                nc.vector.tensor_tensor(xsq[:], xb[:], xb[:], Op.mult)
                ln_state[t] = (xb, xsq)

            def ln_B(t):
                xb, xsq = ln_state[t]
                g = lnps_pool.tile([128, 1024], dt, tag="ln")
                nc.tensor.matmul(g[0:1, 0:512], onesC[:, 0:1], xb[:])
                nc.tensor.matmul(g[0:1, 512:1024], onesC[:, 0:1], xsq[:])
                mcp = sm_pool.tile([1, 512], dt, tag="mcp")
                msq = sm_pool.tile([1, 512], dt, tag="msq")
                var = sm_pool.tile([1, 512], dt, tag="var")
                nc.vector.tensor_copy(mcp[:], g[0:1, 0:512])
                nc.vector.tensor_tensor(msq[:], mcp[:], mcp[:], Op.mult)
                nc.vector.scalar_tensor_tensor(
                    var[:], g[0:1, 512:1024], 1.0, msq[:], Op.mult, Op.subtract
                )
                ln_state[t] = (mcp, var)

            def ln_C(t):
                mcp, var = ln_state[t]
                lnv = sm_pool.tile([1, 512], dt, tag="lnv")
                ru = sm_pool.tile([1, 1024], dt, tag="ru")
                nc.scalar.activation(lnv[:], var[:], F.Ln, bias=epsc[0:1, 0:1])
                nc.scalar.activation(ru[0:1, 0:512], lnv[:], F.Exp, scale=-0.5)
                nc.vector.tensor_tensor(
                    ru[0:1, 512:1024], mcp[:], ru[0:1, 0:512], Op.mult
                )
                ln_state[t] = ru

            def ln_D(t):
                sl = ts(t, 512)
                ru = ln_state.pop(t)
                # broadcast rstd|u over partitions via hi+lo bf16 matmuls:
                # bf16 streams 1 col/cycle (f32 is 4x slower) and the hi+lo
                # accumulation keeps ~fp32 precision.
                ruh = sm_pool.tile([1, 1024], dtb, tag="ruh")
                rul = sm_pool.tile([1, 1024], dtb, tag="rul")
                nc.vector.tensor_copy(ruh[:], ru[:])
                nc.vector.tensor_tensor(rul[:], ru[:], ruh[:], Op.subtract)
                bc = lnps_pool.tile([128, 1024], dt, tag="ln")
                for half in (0, 1):
                    hs = slice(512 * half, 512 * half + 512)
                    nc.tensor.matmul(
                        bc[:, hs], ones1[0:1, :], ruh[0:1, hs],
                        start=True, stop=False,
                    )
                    nc.tensor.matmul(
                        bc[:, hs], ones1[0:1, :], rul[0:1, hs],
                        start=False, stop=True,
                    )
                tmp = gx_pool.tile([128, 512], dt, tag="xtmp")
                nc.vector.tensor_tensor(tmp[:], x_sb[:, sl], bc[:, 0:512], Op.mult)
                nc.vector.tensor_tensor(
                    xn[:, sl], tmp[:], bc[:, 512:1024], Op.subtract
                )

            def ln_E(t):
                sl = ts(t, 512)
                qk = lnps_pool.tile([128, 1024], dt, tag="ln")
                if t < NIT:
                    nc.tensor.matmul(qk[:, 0:512], wq[:], xn[:, sl])
                    for h in range(HEADS):
                        nc.vector.tensor_scalar(
                            q_pad[h][32 * h : 32 * h + 32, sl],
                            qk[32 * h : 32 * h + 32, 0:512],
                            bq[32 * h : 32 * h + 32, 0:1],
                            None,
                            Op.add,
                        )
                nc.tensor.matmul(qk[:, 512:1024], wk[:], xn[:, sl])
                nc.vector.tensor_copy(k_sb[:, sl], qk[:, 512:1024])
                vp = lnps_pool.tile([128, 1024], dt, tag="ln")
                for c in range(4):
                    jc = 4 * t + c
                    nc.tensor.matmul(
                        vp[:, 128 * c : 128 * c + 128], xn[:, ts(jc, 128)], wv[:]
                    )
                    base = jc * 512
                    dst01 = (
                        vaug[:, base : base + 512]
                        .rearrange("p (g e) -> p g e", e=256)[:, :, 0:32]
                    )
                    src01 = vp[:, 128 * c : 128 * c + 128].rearrange(
                        "p (g e) -> p g e", e=32
                    )[:, 0:2, :]
                    nc.vector.tensor_copy(dst01, src01)
                    dst23 = (
                        vaug[:, base : base + 512]
                        .rearrange("p (g e) -> p g e", e=256)[:, :, 192:224]
                    )
                    src23 = vp[:, 128 * c : 128 * c + 128].rearrange(
                        "p (g e) -> p g e", e=32
                    )[:, 2:4, :]
                    nc.vector.tensor_copy(dst23, src23)

            LN_FNS = (ln_A, ln_B, ln_C, ln_D, ln_E)

            def ln_slot_schedule():
                """jc -> [closures] for s-tiles 1-7 inside i-tile 0; slots
                at jc 4t-8, 4t-6, 4t-4, 4t-2, 4t-1 (clamped; <0 = prologue
                leftovers returned separately)."""
                sched = {}
                pro = []
                for t in range(1, 8):
                    for fn, off in zip(LN_FNS, (-8, -6, -4, -2, -1)):
                        jc = 4 * t + off
                        if jc < 0:
                            pro.append((fn, t))
                        else:
                            sched.setdefault(jc, []).append((fn, t))
                return pro, sched

            # ---------------- attention ----------------
            def make_tail(it, pairS):
                """Normalize+project closures run against the SBUF spill,
                sprinkled into the next i-tile's jc loop (off critical
                path). rec rows: 0 = even-head 1/denom, 32 = odd-head;
                cols 0:512 pairA, 512:1024 pairB."""
                isl = ts(it, TI)
                st = {}

                def t_rec():
                    nc.vector.reciprocal(rec[0:1, :], pairS[32:33, :])
                    nc.vector.reciprocal(rec[32:33, :], pairS[96:97, :])

                def t_catA():
                    bcy = lnps_pool.tile([128, 1024], dt, tag="ln")
                    st["bcy"] = bcy
                    bc = bcy[0:97, 0:512]
                    nc.tensor.matmul(bc, ones2[0:33, :], rec[0:33, 0:512])
                    nc.vector.tensor_tensor(
                        catA[0:33, :], pairS[0:33, 0:512], bc[0:33, :], Op.mult
                    )
                    nc.vector.tensor_tensor(
                        catA[64:97, :], pairS[64:97, 0:512], bc[64:97, :],
                        Op.mult,
                    )

                def t_catB():
                    bcy = st["bcy"]
                    bc = bcy[0:97, 0:512]
                    nc.tensor.matmul(
                        bcy[0:128, 512:1024], woa[:, :], catA[0:97, :],
                        start=True, stop=False,
                    )
                    nc.tensor.matmul(bc, ones2[0:33, :], rec[0:33, 512:1024])
                    nc.vector.tensor_tensor(
                        catB[0:33, :], pairS[0:33, 512:1024], bc[0:33, :],
                        Op.mult,
                    )
                    nc.vector.tensor_tensor(
                        catB[64:97, :], pairS[64:97, 512:1024], bc[64:97, :],
                        Op.mult,
                    )

                def t_y():
                    bcy = st["bcy"]
                    yp = bcy[0:128, 512:1024]
                    nc.tensor.matmul(
                        yp, wob[:, :], catB[0:97, :], start=False, stop=True
                    )
                    ysb = ysb_pool.tile([128, TI], dt, tag="ysb")
                    nc.vector.tensor_scalar(
                        ysb[:], yp, bo[:, 0:1], None, Op.add
                    )
                    nc.sync.dma_start(y_d[:, isl], ysb[:])

                return [t_rec, t_catA, t_catB, t_y]

            def run_it(it, ln_sched, tail_pieces):
                isl = ts(it, TI)
                pairA = pair_pool.tile([128, TI], dt, tag="pair")
                pairB = pair_pool.tile([128, TI], dt, tag="pair")

                def emit_sims(jc):
                    ksl = ts(jc, JCHUNK)
                    duoX = duop.tile([128, 1024], dt, tag="duo")
                    nc.tensor.matmul(duoX[:, 0:512], k_sb[:, ksl], q_pad[0][:, isl])
                    nc.tensor.matmul(duoX[:, 512:1024], k_sb[:, ksl], q_pad[2][:, isl])
                    duoY = duop.tile([128, 1024], dt, tag="duo")
                    nc.tensor.matmul(duoY[:, 0:512], k_sb[:, ksl], q_pad[1][:, isl])
                    nc.tensor.matmul(duoY[:, 512:1024], k_sb[:, ksl], q_pad[3][:, isl])
                    return duoX, duoY

                duoX, duoY = emit_sims(0)
                for jc in range(NJC):
                    st, sp_ = jc == 0, jc == NJC - 1
                    vbase = jc * 512
                    for fn, t in ln_sched.get(jc, ()):
                        fn(t)
                    if tail_pieces and 1 <= jc < 1 + 2 * len(tail_pieces) and jc % 2:
                        tail_pieces[(jc - 1) // 2]()
                    expX = expo_pool.tile([128, 1024], dtb, tag="expo")
                    nc.scalar.activation(expX[:], duoX[:], F.Exp)
                    expY = expo_pool.tile([128, 1024], dtb, tag="expo")
                    nc.scalar.activation(expY[:], duoY[:], F.Exp)
                    if jc + 1 < NJC:
                        duoX, duoY = emit_sims(jc + 1)
                    nc.tensor.matmul(
                        pairA[:, :], vaug[:, vbase : vbase + 128],
                        expX[:, 0:512],
                        start=st, stop=False, skip_group_check=True,
                    )
                    nc.tensor.matmul(
                        pairA[:, :], vaug[:, vbase + 128 : vbase + 256],
                        expX[:, 512:1024],
                        start=False, stop=sp_, skip_group_check=True,
                    )
                    nc.tensor.matmul(
                        pairB[:, :], vaug[:, vbase + 256 : vbase + 384],
                        expY[:, 0:512],
                        start=st, stop=False, skip_group_check=True,
                    )
                    nc.tensor.matmul(
                        pairB[:, :], vaug[:, vbase + 384 : vbase + 512],
                        expY[:, 512:1024],
                        start=False, stop=sp_, skip_group_check=True,
                    )
                # spill pairs to SBUF: frees the pair banks after 2 quick
                # DVE copies; the tail runs later against the copy.
                pairS = spill_pool.tile([128, 1024], dt, tag="pairS")
                nc.vector.tensor_copy(pairS[:, 0:512], pairA[:])
                nc.vector.tensor_copy(pairS[:, 512:1024], pairB[:])
                return make_tail(it, pairS)

            pro, sched = ln_slot_schedule()
            for fn in LN_FNS:
                fn(0)
            for fn, t in pro:
                fn(t)
            tail = run_it(0, sched, None)
            for it in range(1, NIT):
                tail = run_it(it, {}, tail)
            for piece in tail:
                piece()

    nc.compile()
    return nc


def _get_program():
    global _PROGRAM
    if _PROGRAM is None:
        _PROGRAM = _build_program()
    return _PROGRAM


def _prep_inputs(x, g, b, w_qkv, w_out, b_out):
    """Host-side sharding + weight folding. All tiny except x slicing."""
    f32 = np.float32
    x = np.asarray(x, f32).reshape(B, C, S)
    g_ = np.asarray(g, f32).reshape(C)
    b_ = np.asarray(b, f32).reshape(C)
    w_qkv = np.asarray(w_qkv, f32)
    w_out = np.asarray(w_out, f32)
    b_out = np.asarray(b_out, f32)

    import ml_dtypes

    bf16 = ml_dtypes.bfloat16
    scale = DIM_HEAD ** -0.5
    wg = w_qkv * g_[None, :]
    bias_qkv = w_qkv @ b_
    hid = HEADS * DIM_HEAD  # 128
    wq_t = np.ascontiguousarray((wg[0:hid] * scale).T).astype(bf16)
    wk_t = np.ascontiguousarray(wg[hid : 2 * hid].T).astype(bf16)
    wv_t = np.ascontiguousarray(wg[2 * hid : 3 * hid].T).astype(bf16)
    bias_q = np.ascontiguousarray((bias_qkv[0:hid] * scale).reshape(128, 1))
    # bias_k is dropped: it shifts all logits of a query equally and
    # softmax is shift-invariant (exact). bias_v folds exactly into the
    # output bias (attention rows sum to 1).
    bias_v = bias_qkv[2 * hid : 3 * hid]

    wo_t = w_out.T  # [hd, o]
    wo_a = np.zeros((97, 128), f32)
    wo_b = np.zeros((97, 128), f32)
    wo_a[0:32] = wo_t[0:32]     # head 0
    wo_a[64:96] = wo_t[64:96]   # head 2
    wo_b[0:32] = wo_t[32:64]    # head 1
    wo_b[64:96] = wo_t[96:128]  # head 3
    bias_o = np.ascontiguousarray((b_out + w_out @ bias_v).reshape(128, 1))

    shared = {
        "wq_t": wq_t,
        "wk_t": wk_t,
        "wv_t": wv_t,
        "wo_a": wo_a,
        "wo_b": wo_b,
        "bias_q": bias_q,
        "bias_o": bias_o,
        "zeros": np.zeros((128, 4096), bf16),
    }
    in_maps = []
    for core in range(N_CORES):
        bb, half = core // 2, core % 2
        if half == 0:
            xc = x[bb]
        else:
            xc = np.concatenate([x[bb][:, HALF:], x[bb][:, :HALF]], axis=1)
        m = {"x": np.ascontiguousarray(xc)}
        m.update(shared)
        in_maps.append(m)
    return in_maps


def _run(inputs, trace=False):
    from concourse.bass_utils import run_bass_kernel_spmd

    nc = _get_program()
    in_maps = _prep_inputs(**inputs)
    res = run_bass_kernel_spmd(
        nc, in_maps, core_ids=list(range(N_CORES)), trace=trace
    )
    y = np.empty((B, C, S), np.float32)
    for core in range(N_CORES):
        bb, half = core // 2, core % 2
        yc = res.results[core]["y"]
        if half == 0:
            y[bb][:, :HALF] = yc
        else:
            y[bb][:, HALF:] = yc
    return y.reshape(B, C, H, W), res


def kernel(x, g, b, w_qkv, w_out, b_out):
    out, _ = _run(
        {"x": x, "g": g, "b": b, "w_qkv": w_qkv, "w_out": w_out, "b_out": b_out}
    )
    return out


# revision 34
# speedup vs baseline: 1.3970x; 1.0706x over previous
"""Trainium2 Bass kernel for nn_Attention_10282151707309.

Reference computation:
  - channel LayerNorm over C=128 (biased var, eps=1e-5, affine g/b)
  - qkv = w_qkv @ xn (1x1 conv), 4 heads x 32 dims, q scaled by 1/sqrt(32)
  - full softmax attention over HW=4096 positions per (batch, head)
  - out = w_out @ attn_out + b_out

Sharding: 8 cores = (batch b in 0..3) x (spatial half in 0..1); each core
runs an identical program on its batch slice (spatially rolled so its own
2048 query columns are program-columns 0:2048 -- softmax is permutation-
equivariant over keys). No collectives; disjoint output slices.

Performance model (from perfetto traces of earlier versions):
  - The ACT engine is the wall: exp of 33.5M sim values at 128 lanes
    @1.2GHz, ~(N+390)/1.2 ns per N-element ACTIVATE. Steady state
    achieves one [128,1024] exp per ~1.2us; everything else must stay
    off ACT and off the exp stream's critical path.
  - PE streams full-mode (128,128)-tile bf16 matmuls at ~215-258ns per
    512 columns with LDWEIGHTS hidden; partial-tile matmuls are ~1.6x
    slower, so sim/av use only full [128,x] operands.
  - Engine queues are strictly in-order: any op whose producers are not
    long-finished head-of-line blocks its whole engine. All cross-engine
    chains (LN, tails) are therefore emission-scheduled several jc
    iterations after their producers.
Structure:
  - One activation-table preload (natural_log_exp_and_others) so Ln/Exp
    never swap tables (the original baseline lost 52us to 41 loads).
  - LN + projections are emission-interleaved under i-tile 0's attention
    in 5 slots per s-tile (xb/xsq -> stats -> rstd -> bc/xn -> proj),
    each ~2 jc after its producers. s-tiles 0-1 partially in prologue.
  - sim full-mode without a padded k: lhsT is the whole k_sb chunk (all
    4 heads' rows), rhs a per-head ZERO-PADDED q copy (zeros kill the
    other heads' k rows exactly). q pads and the zero-padded av lhsT
    tiles (vaug) are zero-filled by DMA from a host zeros tensor --
    no big on-chip memsets (gpsimd per-op overhead is ~1.2-2.2us).
  - k-bias dropped entirely (softmax shift-invariance, exact); v-bias
    folded into the output bias; g/b folded into the qkv weights.
  - i-tile tails are taken OFF the critical path: pairs are spilled
    PSUM->SBUF (2 DVE copies) at it end, which immediately frees the
    pair banks for the next i-tile; the normalize/project tail runs
    against the SBUF copy, sprinkled into the next i-tile's jc loop.
PSUM (8 banks): lnps [128,1024] (2; LN generations + tail bc|y)
  + duo 2x[128,1024] (4) + pairs 2x[128,512] (2).
"""

import numpy as np

HEADS = 4
DIM_HEAD = 32
B, C, H, W = 4, 128, 64, 64
S = H * W              # 4096 spatial positions
HALF = S // 2          # 2048 own query columns per core
TI = 512               # i-tile (query) size
NIT = HALF // TI       # 4 i-tiles
JCHUNK = 128           # j-chunk (key) size
NJC = S // JCHUNK      # 32 j-chunks
EPS = 1e-5
N_CORES = 8

_PROGRAM = None


def _build_program():
    """Build the (SPMD-identical) Bass program once per process."""
    import concourse.bass as bass  # noqa: F401
    import concourse.mybir as mybir
    import concourse.tile as tile
    from concourse import bacc
    from concourse.bass import ts

    dt = mybir.dt.float32
    dtr = mybir.dt.float32r
    dtb = mybir.dt.bfloat16
    F = mybir.ActivationFunctionType
    Op = mybir.AluOpType

    nc = bacc.Bacc(
        "TRN2",
        target_bir_lowering=False,
        debug=False,
        num_devices=N_CORES,
    )

    x_d = nc.dram_tensor("x", [C, S], dt, kind="ExternalInput").ap()
    wq_d = nc.dram_tensor("wq_t", [C, 128], dtb, kind="ExternalInput").ap()
    wk_d = nc.dram_tensor("wk_t", [C, 128], dtb, kind="ExternalInput").ap()
    wv_d = nc.dram_tensor("wv_t", [C, 128], dtb, kind="ExternalInput").ap()
    woa_d = nc.dram_tensor("wo_a", [97, 128], dt, kind="ExternalInput").ap()
    wob_d = nc.dram_tensor("wo_b", [97, 128], dt, kind="ExternalInput").ap()
    bq_d = nc.dram_tensor("bias_q", [128, 1], dt, kind="ExternalInput").ap()
    bo_d = nc.dram_tensor("bias_o", [128, 1], dt, kind="ExternalInput").ap()
    z_d = nc.dram_tensor("zeros", [128, 4096], dtb, kind="ExternalInput").ap()
    y_d = nc.dram_tensor("y", [C, HALF], dt, kind="ExternalOutput").ap()

    with tile.TileContext(nc) as tc:
        from contextlib import ExitStack

        with ExitStack() as ctx:
            const_pool = ctx.enter_context(tc.tile_pool(name="const", bufs=1))
            big_pool = ctx.enter_context(tc.tile_pool(name="big", bufs=1))

            # One table set (natural_log_exp_and_others, id 6) serves every
            # activation in this kernel (Exp, Ln); preload it once.
            nc.scalar.add_instruction(
                mybir.InstLoadActFuncSet(
                    name="act_preload", act_func_set_id=6, ins=[], outs=[]
                )
            )

            wq = const_pool.tile([C, 128], dtb, tag="wq")
            wk = const_pool.tile([C, 128], dtb, tag="wk")
            wv = const_pool.tile([C, 128], dtb, tag="wv")
            woa = const_pool.tile([97, 128], dt, tag="woa")
            wob = const_pool.tile([97, 128], dt, tag="wob")
            bq = const_pool.tile([128, 1], dt, tag="bq")
            bo = const_pool.tile([128, 1], dt, tag="bo")
            ones1 = const_pool.tile([1, 128], dtb, tag="ones1")
            onesC = const_pool.tile([128, 1], dtb, tag="onesC")
            # bc lhsT: row 0 -> out rows 0:33 (even-head reciprocal), row 32
            # -> out rows 64:97 (odd-head). Engine AP partition bases must be
            # 32-aligned, so the two reciprocal rows sit at partitions 0/32.
            ones2 = const_pool.tile([33, 97], dt, tag="ones2")
            epsc = const_pool.tile([1, 1], dt, tag="epsc")

            x_sb = big_pool.tile([C, S], dt, tag="x")
            xn = big_pool.tile([C, S], dtb, tag="xn")
            k_sb = big_pool.tile([128, S], dtb, tag="k")
            # vaug: per j-chunk four [128, 128] full-mode av lhsT tiles in
            # order [h0, h2, h1, h3]; h0/h1 carry (v^T | ones) at cols 0-32,
            # h2/h3 at cols 64-96, everything else zero.
            vaug = big_pool.tile([128, NJC * 512], dtb, tag="vaug")
            q_pad = [
                big_pool.tile(
                    [128, HALF], dtb, tag=f"qpad{h}", name=f"qpad{h}"
                )
                for h in range(HEADS)
            ]
            catA = big_pool.tile([128, TI], dt, tag="catA")
            catB = big_pool.tile([128, TI], dt, tag="catB")
            rec = big_pool.tile([64, 1024], dt, tag="rec")

            # input DMAs; zero fills come from the host zeros tensor so no
            # engine spends time on them.
            nc.sync.dma_start(x_sb[:, 0:512], x_d[:, 0:512])
            nc.sync.dma_start(wq[:], wq_d[:])
            nc.sync.dma_start(wk[:], wk_d[:])
            nc.sync.dma_start(wv[:], wv_d[:])
            nc.sync.dma_start(woa[:], woa_d[:])
            nc.sync.dma_start(wob[:], wob_d[:])
            nc.sync.dma_start(bq[:], bq_d[:])
            nc.sync.dma_start(bo[:], bo_d[:])
            nc.sync.dma_start(
                vaug[:, 0:4096], z_d[:, 0:4096]
            )
            for h in range(HEADS):
                nc.sync.dma_start(q_pad[h][:], z_d[:, 0:HALF])
            for t in range(1, 4):
                nc.sync.dma_start(x_sb[:, ts(t, 512)], x_d[:, ts(t, 512)])
            for i in range(1, 4):
                nc.sync.dma_start(
                    vaug[:, 4096 * i : 4096 * (i + 1)], z_d[:, 0:4096]
                )
                if 3 + i < 8:
                    nc.sync.dma_start(
                        x_sb[:, ts(3 + i, 512)], x_d[:, ts(3 + i, 512)]
                    )
            nc.sync.dma_start(x_sb[:, ts(7, 512)], x_d[:, ts(7, 512)])

            nc.vector.memset(ones1[:], 1.0)
            nc.vector.memset(onesC[:], 1.0 / C)
            nc.vector.memset(ones2[:], 0.0)
            nc.vector.memset(ones2[0:1, 0:33], 1.0)
            nc.vector.memset(ones2[32:33, 64:97], 1.0)
            nc.vector.memset(rec[:], 0.0)
            nc.vector.memset(epsc[:], EPS)
            # cat rows 33:64 are read by the K=97 y matmul (against zero
            # rows of wo) -- zero once so they are never NaN. Row 32 is
            # rewritten by every tail (32-aligned memset base).
            nc.vector.memset(catA[32:64, :], 0.0)
            nc.vector.memset(catB[32:64, :], 0.0)

            def vaug_ones(i):
                """Softmax-denominator ones columns of vaug chunk i (8 jc):
                col 32 of the h0/h1 tiles, col 96 of h2/h3. Emitted per
                chunk so each waits only its own zero-DMA."""
                blk = vaug[:, 4096 * i : 4096 * (i + 1)].rearrange(
                    "p (c g e) -> p c g e", g=2, e=256
                )
                nc.vector.memset(blk[:, :, :, 32:33], 1.0)
                nc.vector.memset(blk[:, :, :, 224:225], 1.0)

            pair_pool = ctx.enter_context(
                tc.tile_pool(name="pair_ps", bufs=2, space="PSUM")
            )
            lnps_pool = ctx.enter_context(
                tc.tile_pool(name="lnps", bufs=1, space="PSUM")
            )
            duop = ctx.enter_context(
                tc.tile_pool(name="duo", bufs=2, space="PSUM")
            )
            expo_pool = ctx.enter_context(tc.tile_pool(name="expo", bufs=6))
            spill_pool = ctx.enter_context(tc.tile_pool(name="spill", bufs=2))
            ysb_pool = ctx.enter_context(tc.tile_pool(name="ysb", bufs=2))
            sm_pool = ctx.enter_context(tc.tile_pool(name="lnsm", bufs=2))
            gx_pool = ctx.enter_context(tc.tile_pool(name="lngx", bufs=3))

            # ---------------- LayerNorm + projections ----------------
            # Five emission slots per s-tile; in i-tile 0 they sit ~2 jc
            # after their producers so no engine head-of-line blocks.
            ln_state = {}

            def ln_A(t):
                sl = ts(t, 512)
                xb = gx_pool.tile([128, 512], dtb, tag="xb")
                xsq = gx_pool.tile([128, 512], dtb, tag="xsq")
                nc.vector.tensor_copy(xb[:], x_sb[:, sl])
                nc.vector.tensor_tensor(xsq[:], xb[:], xb[:], Op.mult)
                ln_state[t] = (xb, xsq)

            def ln_B(t):
                xb, xsq = ln_state[t]
                g = lnps_pool.tile([128, 1024], dt, tag="ln")
                nc.tensor.matmul(g[0:1, 0:512], onesC[:, 0:1], xb[:])
                nc.tensor.matmul(g[0:1, 512:1024], onesC[:, 0:1], xsq[:])
                mcp = sm_pool.tile([1, 512], dt, tag="mcp")
                msq = sm_pool.tile([1, 512], dt, tag="msq")
                var = sm_pool.tile([1, 512], dt, tag="var")
                nc.vector.tensor_copy(mcp[:], g[0:1, 0:512])
                nc.vector.tensor_tensor(msq[:], mcp[:], mcp[:], Op.mult)
                nc.vector.scalar_tensor_tensor(
                    var[:], g[0:1, 512:1024], 1.0, msq[:], Op.mult, Op.subtract
                )
                ln_state[t] = (mcp, var)

            def ln_C(t):
                mcp, var = ln_state[t]
                lnv = sm_pool.tile([1, 512], dt, tag="lnv")
                ru = sm_pool.tile([1, 1024], dt, tag="ru")
                nc.scalar.activation(lnv[:], var[:], F.Ln, bias=epsc[0:1, 0:1])
                nc.scalar.activation(ru[0:1, 0:512], lnv[:], F.Exp, scale=-0.5)
                nc.vector.tensor_tensor(
                    ru[0:1, 512:1024], mcp[:], ru[0:1, 0:512], Op.mult
                )
                ln_state[t] = ru

            def ln_D(t):
                sl = ts(t, 512)
                ru = ln_state.pop(t)
                # broadcast rstd|u over partitions via hi+lo bf16 matmuls:
                # bf16 streams 1 col/cycle (f32 is 4x slower) and the hi+lo
                # accumulation keeps ~fp32 precision.
                ruh = sm_pool.tile([1, 1024], dtb, tag="ruh")
                rul = sm_pool.tile([1, 1024], dtb, tag="rul")
                nc.vector.tensor_copy(ruh[:], ru[:])
                nc.vector.tensor_tensor(rul[:], ru[:], ruh[:], Op.subtract)
                bc = lnps_pool.tile([128, 1024], dt, tag="ln")
                for half in (0, 1):
                    hs = slice(512 * half, 512 * half + 512)
                    nc.tensor.matmul(
                        bc[:, hs], ones1[0:1, :], ruh[0:1, hs],
                        start=True, stop=False,
                    )
                    nc.tensor.matmul(
                        bc[:, hs], ones1[0:1, :], rul[0:1, hs],
                        start=False, stop=True,
                    )
                tmp = gx_pool.tile([128, 512], dt, tag="xtmp")
                nc.vector.tensor_tensor(tmp[:], x_sb[:, sl], bc[:, 0:512], Op.mult)
                nc.vector.tensor_tensor(
                    xn[:, sl], tmp[:], bc[:, 512:1024], Op.subtract
                )

            def ln_E1(t):
                sl = ts(t, 512)
                qk = lnps_pool.tile([128, 1024], dt, tag="ln")
                if t < NIT:
                    nc.tensor.matmul(qk[:, 0:512], wq[:], xn[:, sl])
                    for h in range(HEADS):
                        nc.vector.tensor_scalar(
                            q_pad[h][32 * h : 32 * h + 32, sl],
                            qk[32 * h : 32 * h + 32, 0:512],
                            bq[32 * h : 32 * h + 32, 0:1],
                            None,
                            Op.add,
                        )
                nc.tensor.matmul(qk[:, 512:1024], wk[:], xn[:, sl])
                nc.vector.tensor_copy(k_sb[:, sl], qk[:, 512:1024])

            def ln_E2(t):
                sl = ts(t, 512)
                vp = lnps_pool.tile([128, 1024], dt, tag="ln")
                for c in range(4):
                    jc = 4 * t + c
                    nc.tensor.matmul(
                        vp[:, 128 * c : 128 * c + 128], xn[:, ts(jc, 128)], wv[:]
                    )
                    base = jc * 512
                    dst01 = (
                        vaug[:, base : base + 512]
                        .rearrange("p (g e) -> p g e", e=256)[:, :, 0:32]
                    )
                    src01 = vp[:, 128 * c : 128 * c + 128].rearrange(
                        "p (g e) -> p g e", e=32
                    )[:, 0:2, :]
                    nc.vector.tensor_copy(dst01, src01)
                    dst23 = (
                        vaug[:, base : base + 512]
                        .rearrange("p (g e) -> p g e", e=256)[:, :, 192:224]
                    )
                    src23 = vp[:, 128 * c : 128 * c + 128].rearrange(
                        "p (g e) -> p g e", e=32
                    )[:, 2:4, :]
                    nc.vector.tensor_copy(dst23, src23)

            LN_FNS = (ln_A, ln_B, ln_C, ln_D, ln_E1, ln_E2)
            LN_OFFS = (-8, -6, -4, -3, -2, -1)

            def ln_slot_schedule():
                """jc -> [closures] for s-tiles 1-7 inside i-tile 0 (<0 =
                prologue leftovers returned separately), plus the vaug
                ones-column memsets for chunks 1-3."""
                sched = {}
                pro = []
                for t in range(1, 8):
                    for fn, off in zip(LN_FNS, LN_OFFS):
                        jc = 4 * t + off
                        if jc < 0:
                            pro.append((fn, t))
                        else:
                            sched.setdefault(jc, []).append((fn, t))
                for i in range(1, 4):
                    sched.setdefault(2 * i - 1, []).append((vaug_ones, i))
                return pro, sched

            # ---------------- attention ----------------
            def make_tail(it, pairS, on_act):
                """Normalize+project closures run against the SBUF spill,
                sprinkled into the next i-tile's jc loop (off critical
                path). rec rows: 0 = even-head 1/denom, 32 = odd-head;
                cols 0:512 pairA, 512:1024 pairB. The final i-tile has
                nothing to hide under, so its reciprocals run as ACT
                ln/exp (~4x lower latency than DVE InstReciprocal)."""
                isl = ts(it, TI)
                st = {}

                def t_rec():
                    if on_act:
                        lnt = sm_pool.tile([1, 1024], dt, tag="lnt")
                        lnt2 = sm_pool.tile([1, 1024], dt, tag="lnt2")
                        nc.scalar.activation(lnt[:], pairS[32:33, :], F.Ln)
                        nc.scalar.activation(
                            rec[0:1, :], lnt[:], F.Exp, scale=-1.0
                        )
                        nc.scalar.activation(lnt2[:], pairS[96:97, :], F.Ln)
                        nc.scalar.activation(
                            rec[32:33, :], lnt2[:], F.Exp, scale=-1.0
                        )
                    else:
                        nc.vector.reciprocal(rec[0:1, :], pairS[32:33, :])
                        nc.vector.reciprocal(rec[32:33, :], pairS[96:97, :])

                def t_catA():
                    bcy = lnps_pool.tile([128, 1024], dt, tag="ln")
                    st["bcy"] = bcy
                    bc = bcy[0:97, 0:512]
                    nc.tensor.matmul(bc, ones2[0:33, :], rec[0:33, 0:512])
                    nc.vector.tensor_tensor(
                        catA[0:33, :], pairS[0:33, 0:512], bc[0:33, :], Op.mult
                    )
                    nc.vector.tensor_tensor(
                        catA[64:97, :], pairS[64:97, 0:512], bc[64:97, :],
                        Op.mult,
                    )

                def t_catB():
                    bcy = st["bcy"]
                    bc = bcy[0:97, 0:512]
                    nc.tensor.matmul(
                        bcy[0:128, 512:1024], woa[:, :], catA[0:97, :],
                        start=True, stop=False,
                    )
                    nc.tensor.matmul(bc, ones2[0:33, :], rec[0:33, 512:1024])
                    nc.vector.tensor_tensor(
                        catB[0:33, :], pairS[0:33, 512:1024], bc[0:33, :],
                        Op.mult,
                    )
                    nc.vector.tensor_tensor(
                        catB[64:97, :], pairS[64:97, 512:1024], bc[64:97, :],
                        Op.mult,
                    )

                def t_y():
                    bcy = st["bcy"]
                    yp = bcy[0:128, 512:1024]
                    nc.tensor.matmul(
                        yp, wob[:, :], catB[0:97, :], start=False, stop=True
                    )
                    ysb = ysb_pool.tile([128, TI], dt, tag="ysb")
                    nc.vector.tensor_scalar(
                        ysb[:], yp, bo[:, 0:1], None, Op.add
                    )
                    nc.sync.dma_start(y_d[:, isl], ysb[:])

                return [t_rec, t_catA, t_catB, t_y]

            def run_it(it, ln_sched, tail_pieces):
                isl = ts(it, TI)
                pairA = pair_pool.tile([128, TI], dt, tag="pair")
                pairB = pair_pool.tile([128, TI], dt, tag="pair")

                def emit_sims(jc):
                    ksl = ts(jc, JCHUNK)
                    duoX = duop.tile([128, 1024], dt, tag="duo")
                    nc.tensor.matmul(duoX[:, 0:512], k_sb[:, ksl], q_pad[0][:, isl])
                    nc.tensor.matmul(duoX[:, 512:1024], k_sb[:, ksl], q_pad[2][:, isl])
                    duoY = duop.tile([128, 1024], dt, tag="duo")
                    nc.tensor.matmul(duoY[:, 0:512], k_sb[:, ksl], q_pad[1][:, isl])
                    nc.tensor.matmul(duoY[:, 512:1024], k_sb[:, ksl], q_pad[3][:, isl])
                    return duoX, duoY

                duoX, duoY = emit_sims(0)
                for jc in range(NJC):
                    st, sp_ = jc == 0, jc == NJC - 1
                    vbase = jc * 512
                    for fn, t in ln_sched.get(jc, ()):
                        fn(t)
                    # tail pieces of the previous i-tile: reciprocals at
                    # jc1, the PE-visible bc/y work only after the ~13us
                    # DVE reciprocal latency has surely passed (jc11+),
                    # so nothing head-of-line blocks the PE.
                    if tail_pieces and jc in (1, 11, 13, 15):
                        tail_pieces[(1, 11, 13, 15).index(jc)]()
                    expX = expo_pool.tile([128, 1024], dtb, tag="expo")
                    nc.scalar.activation(expX[:], duoX[:], F.Exp)
                    expY = expo_pool.tile([128, 1024], dtb, tag="expo")
                    nc.scalar.activation(expY[:], duoY[:], F.Exp)
                    if jc + 1 < NJC:
                        duoX, duoY = emit_sims(jc + 1)
                    nc.tensor.matmul(
                        pairA[:, :], vaug[:, vbase : vbase + 128],
                        expX[:, 0:512],
                        start=st, stop=False, skip_group_check=True,
                    )
                    nc.tensor.matmul(
                        pairA[:, :], vaug[:, vbase + 128 : vbase + 256],
                        expX[:, 512:1024],
                        start=False, stop=sp_, skip_group_check=True,
                    )
                    nc.tensor.matmul(
                        pairB[:, :], vaug[:, vbase + 256 : vbase + 384],
                        expY[:, 0:512],
                        start=st, stop=False, skip_group_check=True,
                    )
                    nc.tensor.matmul(
                        pairB[:, :], vaug[:, vbase + 384 : vbase + 512],
                        expY[:, 512:1024],
                        start=False, stop=sp_, skip_group_check=True,
                    )
                # spill pairs to SBUF: frees the pair banks after 2 quick
                # DVE copies; the tail runs later against the copy.
                pairS = spill_pool.tile([128, 1024], dt, tag="pairS")
                nc.vector.tensor_copy(pairS[:, 0:512], pairA[:])
                nc.vector.tensor_copy(pairS[:, 512:1024], pairB[:])
                return make_tail(it, pairS, on_act=it == NIT - 1)

            pro, sched = ln_slot_schedule()
            vaug_ones(0)
            for fn in LN_FNS:
                fn(0)
            for fn, t in pro:
                fn(t)
            tail = run_it(0, sched, None)
            for it in range(1, NIT):
                tail = run_it(it, {}, tail)
            for piece in tail:
                piece()

    nc.compile()
    return nc


def _get_program():
    global _PROGRAM
    if _PROGRAM is None:
        _PROGRAM = _build_program()
    return _PROGRAM


def _prep_inputs(x, g, b, w_qkv, w_out, b_out):
    """Host-side sharding + weight folding. All tiny except x slicing."""
    f32 = np.float32
    x = np.asarray(x, f32).reshape(B, C, S)
    g_ = np.asarray(g, f32).reshape(C)
    b_ = np.asarray(b, f32).reshape(C)
    w_qkv = np.asarray(w_qkv, f32)
    w_out = np.asarray(w_out, f32)
    b_out = np.asarray(b_out, f32)

    import ml_dtypes

    bf16 = ml_dtypes.bfloat16
    scale = DIM_HEAD ** -0.5
    wg = w_qkv * g_[None, :]
    bias_qkv = w_qkv @ b_
    hid = HEADS * DIM_HEAD  # 128
    wq_t = np.ascontiguousarray((wg[0:hid] * scale).T).astype(bf16)
    wk_t = np.ascontiguousarray(wg[hid : 2 * hid].T).astype(bf16)
    wv_t = np.ascontiguousarray(wg[2 * hid : 3 * hid].T).astype(bf16)
    bias_q = np.ascontiguousarray((bias_qkv[0:hid] * scale).reshape(128, 1))
    # bias_k is dropped: it shifts all logits of a query equally and
    # softmax is shift-invariant (exact). bias_v folds exactly into the
    # output bias (attention rows sum to 1).
    bias_v = bias_qkv[2 * hid : 3 * hid]

    wo_t = w_out.T  # [hd, o]
    wo_a = np.zeros((97, 128), f32)
    wo_b = np.zeros((97, 128), f32)
    wo_a[0:32] = wo_t[0:32]     # head 0
    wo_a[64:96] = wo_t[64:96]   # head 2
    wo_b[0:32] = wo_t[32:64]    # head 1
    wo_b[64:96] = wo_t[96:128]  # head 3
    bias_o = np.ascontiguousarray((b_out + w_out @ bias_v).reshape(128, 1))

    shared = {
        "wq_t": wq_t,
        "wk_t": wk_t,
        "wv_t": wv_t,
        "wo_a": wo_a,
        "wo_b": wo_b,
        "bias_q": bias_q,
        "bias_o": bias_o,
        "zeros": np.zeros((128, 4096), bf16),
    }
    in_maps = []
    for core in range(N_CORES):
        bb, half = core // 2, core % 2
        if half == 0:
            xc = x[bb]
        else:
            xc = np.concatenate([x[bb][:, HALF:], x[bb][:, :HALF]], axis=1)
        m = {"x": np.ascontiguousarray(xc)}
        m.update(shared)
        in_maps.append(m)
    return in_maps


def _run(inputs, trace=False):
    from concourse.bass_utils import run_bass_kernel_spmd

    nc = _get_program()
    in_maps = _prep_inputs(**inputs)
    res = run_bass_kernel_spmd(
        nc, in_maps, core_ids=list(range(N_CORES)), trace=trace
    )
    y = np.empty((B, C, S), np.float32)
    for core in range(N_CORES):
        bb, half = core // 2, core % 2
        yc = res.results[core]["y"]
        if half == 0:
            y[bb][:, :HALF] = yc
        else:
            y[bb][:, HALF:] = yc
    return y.reshape(B, C, H, W), res


def kernel(x, g, b, w_qkv, w_out, b_out):
    out, _ = _run(
        {"x": x, "g": g, "b": b, "w_qkv": w_qkv, "w_out": w_out, "b_out": b_out}
    )
    return out


# revision 35
# speedup vs baseline: 1.4571x; 1.0430x over previous
"""Trainium2 Bass kernel for nn_Attention_10282151707309.

Reference computation:
  - channel LayerNorm over C=128 (biased var, eps=1e-5, affine g/b)
  - qkv = w_qkv @ xn (1x1 conv), 4 heads x 32 dims, q scaled by 1/sqrt(32)
  - full softmax attention over HW=4096 positions per (batch, head)
  - out = w_out @ attn_out + b_out

Sharding: 8 cores = (batch b in 0..3) x (spatial half in 0..1); each core
runs an identical program on its batch slice (spatially rolled so its own
2048 query columns are program-columns 0:2048 -- softmax is permutation-
equivariant over keys). No collectives; disjoint output slices.

Performance model (from perfetto traces of earlier versions):
  - The ACT engine is the wall: exp of 33.5M sim values at 128 lanes
    @1.2GHz, ~(N+390)/1.2 ns per N-element ACTIVATE. Steady state
    achieves one [128,1024] exp per ~1.2us; everything else must stay
    off ACT and off the exp stream's critical path.
  - PE streams full-mode (128,128)-tile bf16 matmuls at ~215-258ns per
    512 columns with LDWEIGHTS hidden; partial-tile matmuls are ~1.6x
    slower, so sim/av use only full [128,x] operands.
  - Engine queues are strictly in-order: any op whose producers are not
    long-finished head-of-line blocks its whole engine. All cross-engine
    chains (LN, tails) are therefore emission-scheduled several jc
    iterations after their producers.
Structure:
  - One activation-table preload (natural_log_exp_and_others) so Ln/Exp
    never swap tables (the original baseline lost 52us to 41 loads).
  - LN + projections are emission-interleaved under i-tile 0's attention
    in 5 slots per s-tile (xb/xsq -> stats -> rstd -> bc/xn -> proj),
    each ~2 jc after its producers. s-tiles 0-1 partially in prologue.
  - sim full-mode without a padded k: lhsT is the whole k_sb chunk (all
    4 heads' rows), rhs a per-head ZERO-PADDED q copy (zeros kill the
    other heads' k rows exactly). q pads and the zero-padded av lhsT
    tiles (vaug) are zero-filled by DMA from a host zeros tensor --
    no big on-chip memsets (gpsimd per-op overhead is ~1.2-2.2us).
  - k-bias dropped entirely (softmax shift-invariance, exact); v-bias
    folded into the output bias; g/b folded into the qkv weights.
  - i-tile tails are taken OFF the critical path: pairs are spilled
    PSUM->SBUF (2 DVE copies) at it end, which immediately frees the
    pair banks for the next i-tile; the normalize/project tail runs
    against the SBUF copy, sprinkled into the next i-tile's jc loop.
PSUM (8 banks): lnps [128,1024] (2; LN generations + tail bc|y)
  + duo 2x[128,1024] (4) + pairs 2x[128,512] (2).
"""

import numpy as np

HEADS = 4
DIM_HEAD = 32
B, C, H, W = 4, 128, 64, 64
S = H * W              # 4096 spatial positions
HALF = S // 2          # 2048 own query columns per core
TI = 512               # i-tile (query) size
NIT = HALF // TI       # 4 i-tiles
JCHUNK = 128           # j-chunk (key) size
NJC = S // JCHUNK      # 32 j-chunks
EPS = 1e-5
N_CORES = 8

_PROGRAM = None


def _build_program():
    """Build the (SPMD-identical) Bass program once per process."""
    import concourse.bass as bass  # noqa: F401
    import concourse.mybir as mybir
    import concourse.tile as tile
    from concourse import bacc
    from concourse.bass import ts

    dt = mybir.dt.float32
    dtr = mybir.dt.float32r
    dtb = mybir.dt.bfloat16
    F = mybir.ActivationFunctionType
    Op = mybir.AluOpType

    nc = bacc.Bacc(
        "TRN2",
        target_bir_lowering=False,
        debug=False,
        num_devices=N_CORES,
    )

    x_d = nc.dram_tensor("x", [C, S], dt, kind="ExternalInput").ap()
    wq_d = nc.dram_tensor("wq_t", [C, 128], dtb, kind="ExternalInput").ap()
    wk_d = nc.dram_tensor("wk_t", [C, 128], dtb, kind="ExternalInput").ap()
    wv_d = nc.dram_tensor("wv_t", [C, 128], dtb, kind="ExternalInput").ap()
    woa_d = nc.dram_tensor("wo_a", [97, 128], dt, kind="ExternalInput").ap()
    wob_d = nc.dram_tensor("wo_b", [97, 128], dt, kind="ExternalInput").ap()
    bq_d = nc.dram_tensor("bias_q", [128, 1], dt, kind="ExternalInput").ap()
    bo_d = nc.dram_tensor("bias_o", [128, 1], dt, kind="ExternalInput").ap()
    z_d = nc.dram_tensor("zeros", [128, 4096], dtb, kind="ExternalInput").ap()
    y_d = nc.dram_tensor("y", [C, HALF], dt, kind="ExternalOutput").ap()

    with tile.TileContext(nc) as tc:
        from contextlib import ExitStack

        with ExitStack() as ctx:
            const_pool = ctx.enter_context(tc.tile_pool(name="const", bufs=1))
            big_pool = ctx.enter_context(tc.tile_pool(name="big", bufs=1))

            # One table set (natural_log_exp_and_others, id 6) serves every
            # activation in this kernel (Exp, Ln); preload it once.
            nc.scalar.add_instruction(
                mybir.InstLoadActFuncSet(
                    name="act_preload", act_func_set_id=6, ins=[], outs=[]
                )
            )

            wq = const_pool.tile([C, 128], dtb, tag="wq")
            wk = const_pool.tile([C, 128], dtb, tag="wk")
            wv = const_pool.tile([C, 128], dtb, tag="wv")
            woa = const_pool.tile([97, 128], dt, tag="woa")
            wob = const_pool.tile([97, 128], dt, tag="wob")
            bq = const_pool.tile([128, 1], dt, tag="bq")
            bo = const_pool.tile([128, 1], dt, tag="bo")
            ones1 = const_pool.tile([1, 128], dtb, tag="ones1")
            onesC = const_pool.tile([128, 1], dtb, tag="onesC")
            # bc lhsT: row 0 -> out rows 0:33 (even-head reciprocal), row 32
            # -> out rows 64:97 (odd-head). Engine AP partition bases must be
            # 32-aligned, so the two reciprocal rows sit at partitions 0/32.
            ones2 = const_pool.tile([33, 97], dt, tag="ones2")
            epsc = const_pool.tile([1, 1], dt, tag="epsc")

            x_sb = big_pool.tile([C, S], dt, tag="x")
            xn = big_pool.tile([C, S], dtb, tag="xn")
            k_sb = big_pool.tile([128, S], dtb, tag="k")
            # vaug: per j-chunk four [128, 128] full-mode av lhsT tiles in
            # order [h0, h2, h1, h3]; h0/h1 carry (v^T | ones) at cols 0-32,
            # h2/h3 at cols 64-96, everything else zero.
            vaug = big_pool.tile([128, NJC * 512], dtb, tag="vaug")
            q_pad = [
                big_pool.tile(
                    [128, HALF], dtb, tag=f"qpad{h}", name=f"qpad{h}"
                )
                for h in range(HEADS)
            ]
            catA = big_pool.tile([128, TI], dt, tag="catA")
            catB = big_pool.tile([128, TI], dt, tag="catB")
            rec = big_pool.tile([64, 1024], dt, tag="rec")

            # input DMAs; zero fills come from the host zeros tensor so no
            # engine spends time on them.
            nc.sync.dma_start(x_sb[:, 0:512], x_d[:, 0:512])
            nc.sync.dma_start(wq[:], wq_d[:])
            nc.sync.dma_start(wk[:], wk_d[:])
            nc.sync.dma_start(wv[:], wv_d[:])
            nc.sync.dma_start(woa[:], woa_d[:])
            nc.sync.dma_start(wob[:], wob_d[:])
            nc.sync.dma_start(bq[:], bq_d[:])
            nc.sync.dma_start(bo[:], bo_d[:])
            nc.sync.dma_start(
                vaug[:, 0:4096], z_d[:, 0:4096]
            )
            for h in range(HEADS):
                nc.sync.dma_start(q_pad[h][:], z_d[:, 0:HALF])
            for t in range(1, 4):
                nc.sync.dma_start(x_sb[:, ts(t, 512)], x_d[:, ts(t, 512)])
            for i in range(1, 4):
                nc.sync.dma_start(
                    vaug[:, 4096 * i : 4096 * (i + 1)], z_d[:, 0:4096]
                )
                if 3 + i < 8:
                    nc.sync.dma_start(
                        x_sb[:, ts(3 + i, 512)], x_d[:, ts(3 + i, 512)]
                    )
            nc.sync.dma_start(x_sb[:, ts(7, 512)], x_d[:, ts(7, 512)])

            nc.vector.memset(ones1[:], 1.0)
            nc.vector.memset(onesC[:], 1.0 / C)
            nc.vector.memset(ones2[:], 0.0)
            nc.vector.memset(ones2[0:1, 0:33], 1.0)
            nc.vector.memset(ones2[32:33, 64:97], 1.0)
            nc.vector.memset(rec[:], 0.0)
            nc.vector.memset(epsc[:], EPS)
            # cat rows 33:64 are read by the K=97 y matmul (against zero
            # rows of wo) -- zero once so they are never NaN. Row 32 is
            # rewritten by every tail (32-aligned memset base).
            nc.vector.memset(catA[32:64, :], 0.0)
            nc.vector.memset(catB[32:64, :], 0.0)

            def vaug_ones(i):
                """Softmax-denominator ones columns of vaug chunk i (8 jc):
                col 32 of the h0/h1 tiles, col 96 of h2/h3. Emitted per
                chunk so each waits only its own zero-DMA."""
                blk = vaug[:, 4096 * i : 4096 * (i + 1)].rearrange(
                    "p (c g e) -> p c g e", g=2, e=256
                )
                nc.vector.memset(blk[:, :, :, 32:33], 1.0)
                nc.vector.memset(blk[:, :, :, 224:225], 1.0)

            pair_pool = ctx.enter_context(
                tc.tile_pool(name="pair_ps", bufs=2, space="PSUM")
            )
            lnps_pool = ctx.enter_context(
                tc.tile_pool(name="lnps", bufs=1, space="PSUM")
            )
            duop = ctx.enter_context(
                tc.tile_pool(name="duo", bufs=2, space="PSUM")
            )
            expo_pool = ctx.enter_context(tc.tile_pool(name="expo", bufs=6))
            spill_pool = ctx.enter_context(tc.tile_pool(name="spill", bufs=2))
            ysb_pool = ctx.enter_context(tc.tile_pool(name="ysb", bufs=2))
            sm_pool = ctx.enter_context(tc.tile_pool(name="lnsm", bufs=2))
            gx_pool = ctx.enter_context(tc.tile_pool(name="lngx", bufs=3))

            # ---------------- LayerNorm + projections ----------------
            # Five emission slots per s-tile; in i-tile 0 they sit ~2 jc
            # after their producers so no engine head-of-line blocks.
            ln_state = {}

            def ln_A(t):
                sl = ts(t, 512)
                xb = gx_pool.tile([128, 512], dtb, tag="xb")
                xsq = gx_pool.tile([128, 512], dtb, tag="xsq")
                nc.vector.tensor_copy(xb[:], x_sb[:, sl])
                nc.vector.tensor_tensor(xsq[:], xb[:], xb[:], Op.mult)
                ln_state[t] = (xb, xsq)

            def ln_B(t):
                xb, xsq = ln_state[t]
                g = lnps_pool.tile([128, 1024], dt, tag="ln")
                nc.tensor.matmul(g[0:1, 0:512], onesC[:, 0:1], xb[:])
                nc.tensor.matmul(g[0:1, 512:1024], onesC[:, 0:1], xsq[:])
                mcp = sm_pool.tile([1, 512], dt, tag="mcp")
                msq = sm_pool.tile([1, 512], dt, tag="msq")
                var = sm_pool.tile([1, 512], dt, tag="var")
                nc.vector.tensor_copy(mcp[:], g[0:1, 0:512])
                nc.vector.tensor_tensor(msq[:], mcp[:], mcp[:], Op.mult)
                nc.vector.scalar_tensor_tensor(
                    var[:], g[0:1, 512:1024], 1.0, msq[:], Op.mult, Op.subtract
                )
                ln_state[t] = (mcp, var)

            def ln_C(t):
                mcp, var = ln_state[t]
                lnv = sm_pool.tile([1, 512], dt, tag="lnv")
                ru = sm_pool.tile([1, 1024], dt, tag="ru")
                nc.scalar.activation(lnv[:], var[:], F.Ln, bias=epsc[0:1, 0:1])
                nc.scalar.activation(ru[0:1, 0:512], lnv[:], F.Exp, scale=-0.5)
                nc.vector.tensor_tensor(
                    ru[0:1, 512:1024], mcp[:], ru[0:1, 0:512], Op.mult
                )
                ln_state[t] = ru

            def ln_D(t):
                sl = ts(t, 512)
                ru = ln_state.pop(t)
                # broadcast rstd|u over partitions via hi+lo bf16 matmuls:
                # bf16 streams 1 col/cycle (f32 is 4x slower) and the hi+lo
                # accumulation keeps ~fp32 precision.
                ruh = sm_pool.tile([1, 1024], dtb, tag="ruh")
                rul = sm_pool.tile([1, 1024], dtb, tag="rul")
                nc.vector.tensor_copy(ruh[:], ru[:])
                nc.vector.tensor_tensor(rul[:], ru[:], ruh[:], Op.subtract)
                bc = lnps_pool.tile([128, 1024], dt, tag="ln")
                for half in (0, 1):
                    hs = slice(512 * half, 512 * half + 512)
                    nc.tensor.matmul(
                        bc[:, hs], ones1[0:1, :], ruh[0:1, hs],
                        start=True, stop=False,
                    )
                    nc.tensor.matmul(
                        bc[:, hs], ones1[0:1, :], rul[0:1, hs],
                        start=False, stop=True,
                    )
                tmp = gx_pool.tile([128, 512], dt, tag="xtmp")
                nc.vector.tensor_tensor(tmp[:], x_sb[:, sl], bc[:, 0:512], Op.mult)
                nc.vector.tensor_tensor(
                    xn[:, sl], tmp[:], bc[:, 512:1024], Op.subtract
                )

            def ln_E1(t):
                sl = ts(t, 512)
                qk = lnps_pool.tile([128, 1024], dt, tag="ln")
                if t < NIT:
                    nc.tensor.matmul(qk[:, 0:512], wq[:], xn[:, sl])
                    for h in range(HEADS):
                        nc.vector.tensor_scalar(
                            q_pad[h][32 * h : 32 * h + 32, sl],
                            qk[32 * h : 32 * h + 32, 0:512],
                            bq[32 * h : 32 * h + 32, 0:1],
                            None,
                            Op.add,
                        )
                nc.tensor.matmul(qk[:, 512:1024], wk[:], xn[:, sl])
                nc.vector.tensor_copy(k_sb[:, sl], qk[:, 512:1024])

            def ln_E2(t):
                sl = ts(t, 512)
                vp = lnps_pool.tile([128, 1024], dt, tag="ln")
                for c in range(4):
                    jc = 4 * t + c
                    nc.tensor.matmul(
                        vp[:, 128 * c : 128 * c + 128], xn[:, ts(jc, 128)], wv[:]
                    )
                    base = jc * 512
                    dst01 = (
                        vaug[:, base : base + 512]
                        .rearrange("p (g e) -> p g e", e=256)[:, :, 0:32]
                    )
                    src01 = vp[:, 128 * c : 128 * c + 128].rearrange(
                        "p (g e) -> p g e", e=32
                    )[:, 0:2, :]
                    nc.vector.tensor_copy(dst01, src01)
                    dst23 = (
                        vaug[:, base : base + 512]
                        .rearrange("p (g e) -> p g e", e=256)[:, :, 192:224]
                    )
                    src23 = vp[:, 128 * c : 128 * c + 128].rearrange(
                        "p (g e) -> p g e", e=32
                    )[:, 2:4, :]
                    nc.vector.tensor_copy(dst23, src23)

            LN_FNS = (ln_A, ln_B, ln_C, ln_D, ln_E1, ln_E2)
            LN_OFFS = (-8, -6, -4, -3, -2, -1)

            def ln_slot_schedule():
                """jc -> [closures] for s-tiles 1-7 inside i-tile 0 (<0 =
                prologue leftovers returned separately), plus the vaug
                ones-column memsets for chunks 1-3."""
                sched = {}
                pro = []
                for t in range(1, 8):
                    for fn, off in zip(LN_FNS, LN_OFFS):
                        jc = 4 * t + off
                        if jc < 0:
                            pro.append((fn, t))
                        else:
                            sched.setdefault(jc, []).append((fn, t))
                for i in range(1, 4):
                    sched.setdefault(2 * i - 1, []).append((vaug_ones, i))
                return pro, sched

            # ---------------- attention ----------------
            def make_tail(it, pairS, on_act):
                """Normalize+project closures run against the SBUF spill,
                sprinkled into the next i-tile's jc loop (off critical
                path). rec rows: 0 = even-head 1/denom, 32 = odd-head;
                cols 0:512 pairA, 512:1024 pairB. The final i-tile has
                nothing to hide under, so its reciprocals run as ACT
                ln/exp (~4x lower latency than DVE InstReciprocal)."""
                isl = ts(it, TI)
                st = {}

                def t_rec():
                    if on_act:
                        lnt = sm_pool.tile([1, 1024], dt, tag="lnt")
                        lnt2 = sm_pool.tile([1, 1024], dt, tag="lnt2")
                        nc.scalar.activation(lnt[:], pairS[32:33, :], F.Ln)
                        nc.scalar.activation(
                            rec[0:1, :], lnt[:], F.Exp, scale=-1.0
                        )
                        nc.scalar.activation(lnt2[:], pairS[96:97, :], F.Ln)
                        nc.scalar.activation(
                            rec[32:33, :], lnt2[:], F.Exp, scale=-1.0
                        )
                    else:
                        nc.vector.reciprocal(rec[0:1, :], pairS[32:33, :])
                        nc.vector.reciprocal(rec[32:33, :], pairS[96:97, :])

                def t_catA():
                    bcy = lnps_pool.tile([128, 1024], dt, tag="ln")
                    st["bcy"] = bcy
                    bc = bcy[0:97, 0:512]
                    nc.tensor.matmul(bc, ones2[0:33, :], rec[0:33, 0:512])
                    nc.vector.tensor_tensor(
                        catA[0:33, :], pairS[0:33, 0:512], bc[0:33, :], Op.mult
                    )
                    nc.vector.tensor_tensor(
                        catA[64:97, :], pairS[64:97, 0:512], bc[64:97, :],
                        Op.mult,
                    )

                def t_catB():
                    bcy = st["bcy"]
                    bc = bcy[0:97, 0:512]
                    nc.tensor.matmul(
                        bcy[0:128, 512:1024], woa[:, :], catA[0:97, :],
                        start=True, stop=False,
                    )
                    nc.tensor.matmul(bc, ones2[0:33, :], rec[0:33, 512:1024])
                    nc.vector.tensor_tensor(
                        catB[0:33, :], pairS[0:33, 512:1024], bc[0:33, :],
                        Op.mult,
                    )
                    nc.vector.tensor_tensor(
                        catB[64:97, :], pairS[64:97, 512:1024], bc[64:97, :],
                        Op.mult,
                    )

                def t_y():
                    bcy = st["bcy"]
                    yp = bcy[0:128, 512:1024]
                    nc.tensor.matmul(
                        yp, wob[:, :], catB[0:97, :], start=False, stop=True
                    )
                    ysb = ysb_pool.tile([128, TI], dt, tag="ysb")
                    nc.vector.tensor_scalar(
                        ysb[:], yp, bo[:, 0:1], None, Op.add
                    )
                    nc.sync.dma_start(y_d[:, isl], ysb[:])

                return [t_rec, t_catA, t_catB, t_y]

            def run_it(it, ln_sched, tail_pieces):
                isl = ts(it, TI)
                pairA = pair_pool.tile([128, TI], dt, tag="pair")
                pairB = pair_pool.tile([128, TI], dt, tag="pair")

                def emit_sims(jc):
                    ksl = ts(jc, JCHUNK)
                    duoX = duop.tile([128, 1024], dt, tag="duo")
                    nc.tensor.matmul(duoX[:, 0:512], k_sb[:, ksl], q_pad[0][:, isl])
                    nc.tensor.matmul(duoX[:, 512:1024], k_sb[:, ksl], q_pad[2][:, isl])
                    duoY = duop.tile([128, 1024], dt, tag="duo")
                    nc.tensor.matmul(duoY[:, 0:512], k_sb[:, ksl], q_pad[1][:, isl])
                    nc.tensor.matmul(duoY[:, 512:1024], k_sb[:, ksl], q_pad[3][:, isl])
                    return duoX, duoY

                duoX, duoY = emit_sims(0)
                for jc in range(NJC):
                    st, sp_ = jc == 0, jc == NJC - 1
                    vbase = jc * 512
                    for fn, t in ln_sched.get(jc, ()):
                        fn(t)
                    # tail pieces of the previous i-tile: reciprocals at
                    # jc1, the PE-visible bc/y work only after the ~13us
                    # DVE reciprocal latency has surely passed (jc11+),
                    # so nothing head-of-line blocks the PE.
                    if tail_pieces and jc in (1, 11, 13, 15):
                        tail_pieces[(1, 11, 13, 15).index(jc)]()
                    expX = expo_pool.tile([128, 1024], dtb, tag="expo")
                    nc.scalar.activation(expX[:], duoX[:], F.Exp)
                    expY = expo_pool.tile([128, 1024], dtb, tag="expo")
                    nc.scalar.activation(expY[:], duoY[:], F.Exp)
                    if jc + 1 < NJC:
                        duoX, duoY = emit_sims(jc + 1)
                    nc.tensor.matmul(
                        pairA[:, :], vaug[:, vbase : vbase + 128],
                        expX[:, 0:512],
                        start=st, stop=False, skip_group_check=True,
                    )
                    nc.tensor.matmul(
                        pairA[:, :], vaug[:, vbase + 128 : vbase + 256],
                        expX[:, 512:1024],
                        start=False, stop=sp_, skip_group_check=True,
                    )
                    nc.tensor.matmul(
                        pairB[:, :], vaug[:, vbase + 256 : vbase + 384],
                        expY[:, 0:512],
                        start=st, stop=False, skip_group_check=True,
                    )
                    nc.tensor.matmul(
                        pairB[:, :], vaug[:, vbase + 384 : vbase + 512],
                        expY[:, 512:1024],
                        start=False, stop=sp_, skip_group_check=True,
                    )
                # spill pairs to SBUF: frees the pair banks after 2 quick
                # DVE copies; the tail runs later against the copy.
                pairS = spill_pool.tile([128, 1024], dt, tag="pairS")
                nc.vector.tensor_copy(pairS[:, 0:512], pairA[:])
                nc.vector.tensor_copy(pairS[:, 512:1024], pairB[:])
                return make_tail(it, pairS, on_act=True)

            pro, sched = ln_slot_schedule()
            vaug_ones(0)
            for fn in LN_FNS:
                fn(0)
            for fn, t in pro:
                fn(t)
            tail = run_it(0, sched, None)
            for it in range(1, NIT):
                tail = run_it(it, {}, tail)
            for piece in tail:
                piece()

    nc.compile()
    return nc


def _get_program():
    global _PROGRAM
    if _PROGRAM is None:
        _PROGRAM = _build_program()
    return _PROGRAM


def _prep_inputs(x, g, b, w_qkv, w_out, b_out):
    """Host-side sharding + weight folding. All tiny except x slicing."""
    f32 = np.float32
    x = np.asarray(x, f32).reshape(B, C, S)
    g_ = np.asarray(g, f32).reshape(C)
    b_ = np.asarray(b, f32).reshape(C)
    w_qkv = np.asarray(w_qkv, f32)
    w_out = np.asarray(w_out, f32)
    b_out = np.asarray(b_out, f32)

    import ml_dtypes

    bf16 = ml_dtypes.bfloat16
    scale = DIM_HEAD ** -0.5
    wg = w_qkv * g_[None, :]
    bias_qkv = w_qkv @ b_
    hid = HEADS * DIM_HEAD  # 128
    wq_t = np.ascontiguousarray((wg[0:hid] * scale).T).astype(bf16)
    wk_t = np.ascontiguousarray(wg[hid : 2 * hid].T).astype(bf16)
    wv_t = np.ascontiguousarray(wg[2 * hid : 3 * hid].T).astype(bf16)
    bias_q = np.ascontiguousarray((bias_qkv[0:hid] * scale).reshape(128, 1))
    # bias_k is dropped: it shifts all logits of a query equally and
    # softmax is shift-invariant (exact). bias_v folds exactly into the
    # output bias (attention rows sum to 1).
    bias_v = bias_qkv[2 * hid : 3 * hid]

    wo_t = w_out.T  # [hd, o]
    wo_a = np.zeros((97, 128), f32)
    wo_b = np.zeros((97, 128), f32)
    wo_a[0:32] = wo_t[0:32]     # head 0
    wo_a[64:96] = wo_t[64:96]   # head 2
    wo_b[0:32] = wo_t[32:64]    # head 1
    wo_b[64:96] = wo_t[96:128]  # head 3
    bias_o = np.ascontiguousarray((b_out + w_out @ bias_v).reshape(128, 1))

    shared = {
        "wq_t": wq_t,
        "wk_t": wk_t,
        "wv_t": wv_t,
        "wo_a": wo_a,
        "wo_b": wo_b,
        "bias_q": bias_q,
        "bias_o": bias_o,
        "zeros": np.zeros((128, 4096), bf16),
    }
    in_maps = []
    for core in range(N_CORES):
        bb, half = core // 2, core % 2
        if half == 0:
            xc = x[bb]
        else:
            xc = np.concatenate([x[bb][:, HALF:], x[bb][:, :HALF]], axis=1)
        m = {"x": np.ascontiguousarray(xc)}
        m.update(shared)
        in_maps.append(m)
    return in_maps


def _run(inputs, trace=False):
    from concourse.bass_utils import run_bass_kernel_spmd

    nc = _get_program()
    in_maps = _prep_inputs(**inputs)
    res = run_bass_kernel_spmd(
        nc, in_maps, core_ids=list(range(N_CORES)), trace=trace
    )
    y = np.empty((B, C, S), np.float32)
    for core in range(N_CORES):
        bb, half = core // 2, core % 2
        yc = res.results[core]["y"]
        if half == 0:
            y[bb][:, :HALF] = yc
        else:
            y[bb][:, HALF:] = yc
    return y.reshape(B, C, H, W), res


def kernel(x, g, b, w_qkv, w_out, b_out):
    out, _ = _run(
        {"x": x, "g": g, "b": b, "w_qkv": w_qkv, "w_out": w_out, "b_out": b_out}
    )
    return out


# revision 38
# speedup vs baseline: 1.4763x; 1.0132x over previous
"""Trainium2 Bass kernel for nn_Attention_10282151707309.

Reference computation:
  - channel LayerNorm over C=128 (biased var, eps=1e-5, affine g/b)
  - qkv = w_qkv @ xn (1x1 conv), 4 heads x 32 dims, q scaled by 1/sqrt(32)
  - full softmax attention over HW=4096 positions per (batch, head)
  - out = w_out @ attn_out + b_out

Sharding: 8 cores = (batch b in 0..3) x (spatial half in 0..1); each core
runs an identical program on its batch slice (spatially rolled so its own
2048 query columns are program-columns 0:2048 -- softmax is permutation-
equivariant over keys). No collectives; disjoint output slices.

Performance model (from perfetto traces of earlier versions):
  - The ACT engine is the wall: exp of 33.5M sim values at 128 lanes
    @1.2GHz, ~(N+390)/1.2 ns per N-element ACTIVATE. Steady state
    achieves one [128,1024] exp per ~1.2us; everything else must stay
    off ACT and off the exp stream's critical path.
  - PE streams full-mode (128,128)-tile bf16 matmuls at ~215-258ns per
    512 columns with LDWEIGHTS hidden; partial-tile matmuls are ~1.6x
    slower, so sim/av use only full [128,x] operands.
  - Engine queues are strictly in-order: any op whose producers are not
    long-finished head-of-line blocks its whole engine. All cross-engine
    chains (LN, tails) are therefore emission-scheduled several jc
    iterations after their producers.
Structure:
  - One activation-table preload (natural_log_exp_and_others) so Ln/Exp
    never swap tables (the original baseline lost 52us to 41 loads).
  - LN + projections are emission-interleaved under i-tile 0's attention
    in 5 slots per s-tile (xb/xsq -> stats -> rstd -> bc/xn -> proj),
    each ~2 jc after its producers. s-tiles 0-1 partially in prologue.
  - sim full-mode without a padded k: lhsT is the whole k_sb chunk (all
    4 heads' rows), rhs a per-head ZERO-PADDED q copy (zeros kill the
    other heads' k rows exactly). q pads and the zero-padded av lhsT
    tiles (vaug) are zero-filled by DMA from a host zeros tensor --
    no big on-chip memsets (gpsimd per-op overhead is ~1.2-2.2us).
  - k-bias dropped entirely (softmax shift-invariance, exact); v-bias
    folded into the output bias; g/b folded into the qkv weights.
  - i-tile tails are taken OFF the critical path: pairs are spilled
    PSUM->SBUF (2 DVE copies) at it end, which immediately frees the
    pair banks for the next i-tile; the normalize/project tail runs
    against the SBUF copy, sprinkled into the next i-tile's jc loop.
PSUM (8 banks): lnps [128,1024] (2; LN generations + tail bc|y)
  + duo 2x[128,1024] (4) + pairs 2x[128,512] (2).
"""

import numpy as np

HEADS = 4
DIM_HEAD = 32
B, C, H, W = 4, 128, 64, 64
S = H * W              # 4096 spatial positions
HALF = S // 2          # 2048 own query columns per core
TI = 512               # i-tile (query) size
NIT = HALF // TI       # 4 i-tiles
JCHUNK = 128           # j-chunk (key) size
NJC = S // JCHUNK      # 32 j-chunks
EPS = 1e-5
N_CORES = 8

_PROGRAM = None


def _build_program():
    """Build the (SPMD-identical) Bass program once per process."""
    import concourse.bass as bass  # noqa: F401
    import concourse.mybir as mybir
    import concourse.tile as tile
    from concourse import bacc
    from concourse.bass import ts

    dt = mybir.dt.float32
    dtr = mybir.dt.float32r
    dtb = mybir.dt.bfloat16
    F = mybir.ActivationFunctionType
    Op = mybir.AluOpType

    nc = bacc.Bacc(
        "TRN2",
        target_bir_lowering=False,
        debug=False,
        num_devices=N_CORES,
    )

    x_d = nc.dram_tensor("x", [C, S], dt, kind="ExternalInput").ap()
    wq_d = nc.dram_tensor("wq_t", [C, 128], dtb, kind="ExternalInput").ap()
    wk_d = nc.dram_tensor("wk_t", [C, 128], dtb, kind="ExternalInput").ap()
    wv_d = nc.dram_tensor("wv_t", [C, 128], dtb, kind="ExternalInput").ap()
    woa_d = nc.dram_tensor("wo_a", [97, 128], dt, kind="ExternalInput").ap()
    wob_d = nc.dram_tensor("wo_b", [97, 128], dt, kind="ExternalInput").ap()
    bq_d = nc.dram_tensor("bias_q", [128, 1], dt, kind="ExternalInput").ap()
    bo_d = nc.dram_tensor("bias_o", [128, 1], dt, kind="ExternalInput").ap()
    z_d = nc.dram_tensor("zeros", [128, 4096], dtb, kind="ExternalInput").ap()
    y_d = nc.dram_tensor("y", [C, HALF], dt, kind="ExternalOutput").ap()

    with tile.TileContext(nc) as tc:
        from contextlib import ExitStack

        with ExitStack() as ctx:
            const_pool = ctx.enter_context(tc.tile_pool(name="const", bufs=1))
            big_pool = ctx.enter_context(tc.tile_pool(name="big", bufs=1))

            # One table set (natural_log_exp_and_others, id 6) serves every
            # activation in this kernel (Exp, Ln); preload it once.
            nc.scalar.add_instruction(
                mybir.InstLoadActFuncSet(
                    name="act_preload", act_func_set_id=6, ins=[], outs=[]
                )
            )

            wq = const_pool.tile([C, 128], dtb, tag="wq")
            wk = const_pool.tile([C, 128], dtb, tag="wk")
            wv = const_pool.tile([C, 128], dtb, tag="wv")
            woa = const_pool.tile([97, 128], dt, tag="woa")
            wob = const_pool.tile([97, 128], dt, tag="wob")
            bq = const_pool.tile([128, 1], dt, tag="bq")
            bo = const_pool.tile([128, 1], dt, tag="bo")
            ones1 = const_pool.tile([1, 128], dtb, tag="ones1")
            onesC = const_pool.tile([128, 1], dtb, tag="onesC")
            # bc lhsT: row 0 -> out rows 0:33 (even-head reciprocal), row 32
            # -> out rows 64:97 (odd-head). Engine AP partition bases must be
            # 32-aligned, so the two reciprocal rows sit at partitions 0/32.
            ones2 = const_pool.tile([33, 97], dt, tag="ones2")
            epsc = const_pool.tile([1, 1], dt, tag="epsc")

            x_sb = big_pool.tile([C, S], dt, tag="x")
            xn = big_pool.tile([C, S], dtb, tag="xn")
            k_sb = big_pool.tile([128, S], dtb, tag="k")
            # vaug: per j-chunk four [128, 128] full-mode av lhsT tiles in
            # order [h0, h2, h1, h3]; h0/h1 carry (v^T | ones) at cols 0-32,
            # h2/h3 at cols 64-96, everything else zero.
            vaug = big_pool.tile([128, NJC * 512], dtb, tag="vaug")
            q_pad = [
                big_pool.tile(
                    [128, HALF], dtb, tag=f"qpad{h}", name=f"qpad{h}"
                )
                for h in range(HEADS)
            ]
            catA = big_pool.tile([128, TI], dt, tag="catA")
            catB = big_pool.tile([128, TI], dt, tag="catB")
            rec = big_pool.tile([64, 1024], dt, tag="rec")

            # input DMAs; zero fills come from the host zeros tensor so no
            # engine spends time on them.
            nc.sync.dma_start(x_sb[:, 0:512], x_d[:, 0:512])
            nc.sync.dma_start(wq[:], wq_d[:])
            nc.sync.dma_start(wk[:], wk_d[:])
            nc.sync.dma_start(wv[:], wv_d[:])
            nc.sync.dma_start(woa[:], woa_d[:])
            nc.sync.dma_start(wob[:], wob_d[:])
            nc.sync.dma_start(bq[:], bq_d[:])
            nc.sync.dma_start(bo[:], bo_d[:])
            nc.sync.dma_start(
                vaug[:, 0:4096], z_d[:, 0:4096]
            )
            for h in range(HEADS):
                nc.sync.dma_start(q_pad[h][:], z_d[:, 0:HALF])
            for t in range(1, 4):
                nc.sync.dma_start(x_sb[:, ts(t, 512)], x_d[:, ts(t, 512)])
            for i in range(1, 4):
                nc.sync.dma_start(
                    vaug[:, 4096 * i : 4096 * (i + 1)], z_d[:, 0:4096]
                )
                if 3 + i < 8:
                    nc.sync.dma_start(
                        x_sb[:, ts(3 + i, 512)], x_d[:, ts(3 + i, 512)]
                    )
            nc.sync.dma_start(x_sb[:, ts(7, 512)], x_d[:, ts(7, 512)])

            nc.vector.memset(ones1[:], 1.0)
            nc.vector.memset(onesC[:], 1.0 / C)
            nc.vector.memset(ones2[:], 0.0)
            nc.vector.memset(ones2[0:1, 0:33], 1.0)
            nc.vector.memset(ones2[32:33, 64:97], 1.0)
            nc.vector.memset(rec[:], 0.0)
            nc.vector.memset(epsc[:], EPS)
            # cat rows 33:64 are read by the K=97 y matmul (against zero
            # rows of wo) -- zero once so they are never NaN. Row 32 is
            # rewritten by every tail (32-aligned memset base).
            nc.vector.memset(catA[32:64, :], 0.0)
            nc.vector.memset(catB[32:64, :], 0.0)

            def vaug_ones(i):
                """Softmax-denominator ones columns of vaug chunk i (8 jc):
                col 32 of the h0/h1 tiles, col 96 of h2/h3. Emitted per
                chunk so each waits only its own zero-DMA."""
                blk = vaug[:, 4096 * i : 4096 * (i + 1)].rearrange(
                    "p (c g e) -> p c g e", g=2, e=256
                )
                nc.vector.memset(blk[:, :, :, 32:33], 1.0)
                nc.vector.memset(blk[:, :, :, 224:225], 1.0)

            pair_pool = ctx.enter_context(
                tc.tile_pool(name="pair_ps", bufs=2, space="PSUM")
            )
            lnps_pool = ctx.enter_context(
                tc.tile_pool(name="lnps", bufs=1, space="PSUM")
            )
            duop = ctx.enter_context(
                tc.tile_pool(name="duo", bufs=2, space="PSUM")
            )
            expo_pool = ctx.enter_context(tc.tile_pool(name="expo", bufs=8))
            spill_pool = ctx.enter_context(tc.tile_pool(name="spill", bufs=2))
            ysb_pool = ctx.enter_context(tc.tile_pool(name="ysb", bufs=2))
            sm_pool = ctx.enter_context(tc.tile_pool(name="lnsm", bufs=2))
            gx_pool = ctx.enter_context(tc.tile_pool(name="lngx", bufs=3))

            # ---------------- LayerNorm + projections ----------------
            # Five emission slots per s-tile; in i-tile 0 they sit ~2 jc
            # after their producers so no engine head-of-line blocks.
            ln_state = {}

            def ln_A(t):
                sl = ts(t, 512)
                xb = gx_pool.tile([128, 512], dtb, tag="xb")
                xsq = gx_pool.tile([128, 512], dtb, tag="xsq")
                nc.vector.tensor_copy(xb[:], x_sb[:, sl])
                nc.vector.tensor_tensor(xsq[:], xb[:], xb[:], Op.mult)
                ln_state[t] = (xb, xsq)

            def ln_B(t):
                xb, xsq = ln_state[t]
                g = lnps_pool.tile([128, 1024], dt, tag="ln")
                nc.tensor.matmul(g[0:1, 0:512], onesC[:, 0:1], xb[:])
                nc.tensor.matmul(g[0:1, 512:1024], onesC[:, 0:1], xsq[:])
                ln_state[t] = g

            def ln_C(t):
                g = ln_state[t]
                mcp = sm_pool.tile([1, 512], dt, tag="mcp")
                msq = sm_pool.tile([1, 512], dt, tag="msq")
                var = sm_pool.tile([1, 512], dt, tag="var")
                lnv = sm_pool.tile([1, 512], dt, tag="lnv")
                ru = sm_pool.tile([1, 1024], dt, tag="ru")
                nc.vector.tensor_copy(mcp[:], g[0:1, 0:512])
                nc.vector.tensor_tensor(msq[:], mcp[:], mcp[:], Op.mult)
                nc.vector.scalar_tensor_tensor(
                    var[:], g[0:1, 512:1024], 1.0, msq[:], Op.mult, Op.subtract
                )
                nc.scalar.activation(lnv[:], var[:], F.Ln, bias=epsc[0:1, 0:1])
                nc.scalar.activation(ru[0:1, 0:512], lnv[:], F.Exp, scale=-0.5)
                ln_state[t] = (mcp, ru)

            def ln_C2(t):
                mcp, ru = ln_state[t]
                # u = mean * rstd; then hi+lo bf16 split of rstd|u for the
                # broadcast matmuls (bf16 streams 1 col/cycle vs f32's 4,
                # hi+lo accumulation keeps ~fp32 precision).
                ruh = sm_pool.tile([1, 1024], dtb, tag="ruh")
                rul = sm_pool.tile([1, 1024], dtb, tag="rul")
                nc.vector.tensor_tensor(
                    ru[0:1, 512:1024], mcp[:], ru[0:1, 0:512], Op.mult
                )
                nc.vector.tensor_copy(ruh[:], ru[:])
                nc.vector.tensor_tensor(rul[:], ru[:], ruh[:], Op.subtract)
                ln_state[t] = (ruh, rul)

            def ln_D(t):
                sl = ts(t, 512)
                ruh, rul = ln_state.pop(t)
                bc = lnps_pool.tile([128, 1024], dt, tag="ln")
                for half in (0, 1):
                    hs = slice(512 * half, 512 * half + 512)
                    nc.tensor.matmul(
                        bc[:, hs], ones1[0:1, :], ruh[0:1, hs],
                        start=True, stop=False,
                    )
                    nc.tensor.matmul(
                        bc[:, hs], ones1[0:1, :], rul[0:1, hs],
                        start=False, stop=True,
                    )
                tmp = gx_pool.tile([128, 512], dt, tag="xtmp")
                nc.vector.tensor_tensor(tmp[:], x_sb[:, sl], bc[:, 0:512], Op.mult)
                nc.vector.tensor_tensor(
                    xn[:, sl], tmp[:], bc[:, 512:1024], Op.subtract
                )
                qk = lnps_pool.tile([128, 1024], dt, tag="ln")
                if t < NIT:
                    nc.tensor.matmul(qk[:, 0:512], wq[:], xn[:, sl])
                    for h in range(HEADS):
                        nc.vector.tensor_scalar(
                            q_pad[h][32 * h : 32 * h + 32, sl],
                            qk[32 * h : 32 * h + 32, 0:512],
                            bq[32 * h : 32 * h + 32, 0:1],
                            None,
                            Op.add,
                        )
                nc.tensor.matmul(qk[:, 512:1024], wk[:], xn[:, sl])
                nc.vector.tensor_copy(k_sb[:, sl], qk[:, 512:1024])

            def ln_E2(t):
                sl = ts(t, 512)
                vp = lnps_pool.tile([128, 1024], dt, tag="ln")
                for c in range(4):
                    jc = 4 * t + c
                    nc.tensor.matmul(
                        vp[:, 128 * c : 128 * c + 128], xn[:, ts(jc, 128)], wv[:]
                    )
                    base = jc * 512
                    dst01 = (
                        vaug[:, base : base + 512]
                        .rearrange("p (g e) -> p g e", e=256)[:, :, 0:32]
                    )
                    src01 = vp[:, 128 * c : 128 * c + 128].rearrange(
                        "p (g e) -> p g e", e=32
                    )[:, 0:2, :]
                    nc.vector.tensor_copy(dst01, src01)
                    dst23 = (
                        vaug[:, base : base + 512]
                        .rearrange("p (g e) -> p g e", e=256)[:, :, 192:224]
                    )
                    src23 = vp[:, 128 * c : 128 * c + 128].rearrange(
                        "p (g e) -> p g e", e=32
                    )[:, 2:4, :]
                    nc.vector.tensor_copy(dst23, src23)

            LN_FNS = (ln_A, ln_B, ln_C, ln_C2, ln_D, ln_E2)
            LN_OFFS = (-8, -6, -5, -3, -2, -1)

            def ln_slot_schedule():
                """jc -> [closures] for s-tiles 1-7 inside i-tile 0 (<0 =
                prologue leftovers returned separately), plus the vaug
                ones-column memsets for chunks 1-3."""
                sched = {}
                pro = []
                for t in range(1, 8):
                    for fn, off in zip(LN_FNS, LN_OFFS):
                        jc = 4 * t + off
                        if jc < 0:
                            pro.append((fn, t))
                        else:
                            sched.setdefault(jc, []).append((fn, t))
                for i in range(1, 4):
                    sched.setdefault(2 * i - 1, []).append((vaug_ones, i))
                return pro, sched

            # ---------------- attention ----------------
            def make_tail(it, pairS, on_act):
                """Normalize+project closures run against the SBUF spill,
                sprinkled into the next i-tile's jc loop (off critical
                path). rec rows: 0 = even-head 1/denom, 32 = odd-head;
                cols 0:512 pairA, 512:1024 pairB. The final i-tile has
                nothing to hide under, so its reciprocals run as ACT
                ln/exp (~4x lower latency than DVE InstReciprocal)."""
                isl = ts(it, TI)
                st = {}

                def t_rec():
                    if on_act:
                        lnt = sm_pool.tile([1, 1024], dt, tag="lnt")
                        lnt2 = sm_pool.tile([1, 1024], dt, tag="lnt2")
                        nc.scalar.activation(lnt[:], pairS[32:33, :], F.Ln)
                        nc.scalar.activation(
                            rec[0:1, :], lnt[:], F.Exp, scale=-1.0
                        )
                        nc.scalar.activation(lnt2[:], pairS[96:97, :], F.Ln)
                        nc.scalar.activation(
                            rec[32:33, :], lnt2[:], F.Exp, scale=-1.0
                        )
                    else:
                        nc.vector.reciprocal(rec[0:1, :], pairS[32:33, :])
                        nc.vector.reciprocal(rec[32:33, :], pairS[96:97, :])

                def t_catA():
                    bcy = lnps_pool.tile([128, 1024], dt, tag="ln")
                    st["bcy"] = bcy
                    bc = bcy[0:97, 0:512]
                    nc.tensor.matmul(bc, ones2[0:33, :], rec[0:33, 0:512])
                    nc.vector.tensor_tensor(
                        catA[0:33, :], pairS[0:33, 0:512], bc[0:33, :], Op.mult
                    )
                    nc.vector.tensor_tensor(
                        catA[64:97, :], pairS[64:97, 0:512], bc[64:97, :],
                        Op.mult,
                    )

                def t_catB():
                    bcy = st["bcy"]
                    bc = bcy[0:97, 0:512]
                    nc.tensor.matmul(
                        bcy[0:128, 512:1024], woa[:, :], catA[0:97, :],
                        start=True, stop=False,
                    )
                    nc.tensor.matmul(bc, ones2[0:33, :], rec[0:33, 512:1024])
                    nc.vector.tensor_tensor(
                        catB[0:33, :], pairS[0:33, 512:1024], bc[0:33, :],
                        Op.mult,
                    )
                    nc.vector.tensor_tensor(
                        catB[64:97, :], pairS[64:97, 512:1024], bc[64:97, :],
                        Op.mult,
                    )

                def t_y():
                    bcy = st["bcy"]
                    yp = bcy[0:128, 512:1024]
                    nc.tensor.matmul(
                        yp, wob[:, :], catB[0:97, :], start=False, stop=True
                    )
                    ysb = ysb_pool.tile([128, TI], dt, tag="ysb")
                    nc.vector.tensor_scalar(
                        ysb[:], yp, bo[:, 0:1], None, Op.add
                    )
                    nc.sync.dma_start(y_d[:, isl], ysb[:])

                return [t_rec, t_catA, t_catB, t_y]

            def run_it(it, ln_sched, tail_pieces):
                isl = ts(it, TI)
                pairA = pair_pool.tile([128, TI], dt, tag="pair")
                pairB = pair_pool.tile([128, TI], dt, tag="pair")

                def emit_sims(jc):
                    ksl = ts(jc, JCHUNK)
                    duoX = duop.tile([128, 1024], dt, tag="duo")
                    nc.tensor.matmul(duoX[:, 0:512], k_sb[:, ksl], q_pad[0][:, isl])
                    nc.tensor.matmul(duoX[:, 512:1024], k_sb[:, ksl], q_pad[2][:, isl])
                    duoY = duop.tile([128, 1024], dt, tag="duo")
                    nc.tensor.matmul(duoY[:, 0:512], k_sb[:, ksl], q_pad[1][:, isl])
                    nc.tensor.matmul(duoY[:, 512:1024], k_sb[:, ksl], q_pad[3][:, isl])
                    return duoX, duoY

                duoX, duoY = emit_sims(0)
                for jc in range(NJC):
                    st, sp_ = jc == 0, jc == NJC - 1
                    vbase = jc * 512
                    for fn, t in ln_sched.get(jc, ()):
                        fn(t)
                    # tail pieces of the previous i-tile: reciprocals at
                    # jc1, the PE-visible bc/y work only after the ~13us
                    # DVE reciprocal latency has surely passed (jc11+),
                    # so nothing head-of-line blocks the PE.
                    if tail_pieces and jc in (1, 11, 13, 15):
                        tail_pieces[(1, 11, 13, 15).index(jc)]()
                    expX = expo_pool.tile([128, 1024], dtb, tag="expo")
                    nc.scalar.activation(expX[:], duoX[:], F.Exp)
                    expY = expo_pool.tile([128, 1024], dtb, tag="expo")
                    nc.scalar.activation(expY[:], duoY[:], F.Exp)
                    if jc + 1 < NJC:
                        duoX, duoY = emit_sims(jc + 1)
                    nc.tensor.matmul(
                        pairA[:, :], vaug[:, vbase : vbase + 128],
                        expX[:, 0:512],
                        start=st, stop=False, skip_group_check=True,
                    )
                    nc.tensor.matmul(
                        pairA[:, :], vaug[:, vbase + 128 : vbase + 256],
                        expX[:, 512:1024],
                        start=False, stop=sp_, skip_group_check=True,
                    )
                    nc.tensor.matmul(
                        pairB[:, :], vaug[:, vbase + 256 : vbase + 384],
                        expY[:, 0:512],
                        start=st, stop=False, skip_group_check=True,
                    )
                    nc.tensor.matmul(
                        pairB[:, :], vaug[:, vbase + 384 : vbase + 512],
                        expY[:, 512:1024],
                        start=False, stop=sp_, skip_group_check=True,
                    )
                # spill pairs to SBUF: frees the pair banks after 2 quick
                # DVE copies; the tail runs later against the copy.
                pairS = spill_pool.tile([128, 1024], dt, tag="pairS")
                nc.vector.tensor_copy(pairS[:, 0:512], pairA[:])
                nc.vector.tensor_copy(pairS[:, 512:1024], pairB[:])
                return make_tail(it, pairS, on_act=True)

            pro, sched = ln_slot_schedule()
            vaug_ones(0)
            for fn in LN_FNS:
                fn(0)
            for fn, t in pro:
                fn(t)
            tail = run_it(0, sched, None)
            for it in range(1, NIT):
                tail = run_it(it, {}, tail)
            for piece in tail:
                piece()

    nc.compile()
    return nc


def _get_program():
    global _PROGRAM
    if _PROGRAM is None:
        _PROGRAM = _build_program()
    return _PROGRAM


def _prep_inputs(x, g, b, w_qkv, w_out, b_out):
    """Host-side sharding + weight folding. All tiny except x slicing."""
    f32 = np.float32
    x = np.asarray(x, f32).reshape(B, C, S)
    g_ = np.asarray(g, f32).reshape(C)
    b_ = np.asarray(b, f32).reshape(C)
    w_qkv = np.asarray(w_qkv, f32)
    w_out = np.asarray(w_out, f32)
    b_out = np.asarray(b_out, f32)

    import ml_dtypes

    bf16 = ml_dtypes.bfloat16
    scale = DIM_HEAD ** -0.5
    wg = w_qkv * g_[None, :]
    bias_qkv = w_qkv @ b_
    hid = HEADS * DIM_HEAD  # 128
    wq_t = np.ascontiguousarray((wg[0:hid] * scale).T).astype(bf16)
    wk_t = np.ascontiguousarray(wg[hid : 2 * hid].T).astype(bf16)
    wv_t = np.ascontiguousarray(wg[2 * hid : 3 * hid].T).astype(bf16)
    bias_q = np.ascontiguousarray((bias_qkv[0:hid] * scale).reshape(128, 1))
    # bias_k is dropped: it shifts all logits of a query equally and
    # softmax is shift-invariant (exact). bias_v folds exactly into the
    # output bias (attention rows sum to 1).
    bias_v = bias_qkv[2 * hid : 3 * hid]

    wo_t = w_out.T  # [hd, o]
    wo_a = np.zeros((97, 128), f32)
    wo_b = np.zeros((97, 128), f32)
    wo_a[0:32] = wo_t[0:32]     # head 0
    wo_a[64:96] = wo_t[64:96]   # head 2
    wo_b[0:32] = wo_t[32:64]    # head 1
    wo_b[64:96] = wo_t[96:128]  # head 3
    bias_o = np.ascontiguousarray((b_out + w_out @ bias_v).reshape(128, 1))

    shared = {
        "wq_t": wq_t,
        "wk_t": wk_t,
        "wv_t": wv_t,
        "wo_a": wo_a,
        "wo_b": wo_b,
        "bias_q": bias_q,
        "bias_o": bias_o,
        "zeros": np.zeros((128, 4096), bf16),
    }
    in_maps = []
    for core in range(N_CORES):
        bb, half = core // 2, core % 2
        if half == 0:
            xc = x[bb]
        else:
            xc = np.concatenate([x[bb][:, HALF:], x[bb][:, :HALF]], axis=1)
        m = {"x": np.ascontiguousarray(xc)}
        m.update(shared)
        in_maps.append(m)
    return in_maps


def _run(inputs, trace=False):
    from concourse.bass_utils import run_bass_kernel_spmd

    nc = _get_program()
    in_maps = _prep_inputs(**inputs)
    res = run_bass_kernel_spmd(
        nc, in_maps, core_ids=list(range(N_CORES)), trace=trace
    )
    y = np.empty((B, C, S), np.float32)
    for core in range(N_CORES):
        bb, half = core // 2, core % 2
        yc = res.results[core]["y"]
        if half == 0:
            y[bb][:, :HALF] = yc
        else:
            y[bb][:, HALF:] = yc
    return y.reshape(B, C, H, W), res


def kernel(x, g, b, w_qkv, w_out, b_out):
    out, _ = _run(
        {"x": x, "g": g, "b": b, "w_qkv": w_qkv, "w_out": w_out, "b_out": b_out}
    )
    return out
